# revision 1
# baseline (speedup 1.0000x reference)
"""RBF kernel ridge regression inference on 8 Trainium2 NeuronCores.

out[q] = sum_t exp(-gamma * ||X[q] - T[t]||^2) * coef[t]

Factored as exp(-g*x2[q]) * sum_t exp(2g*dot[t,q] - g*y2[t]) * coef[t] so the
whole inner computation maps onto TensorE (bf16 GEMM + matvec) and ScalarE
(one fused exp with per-partition bias).  Row norms are computed on DVE
(square+accumulate in one scalar_tensor_tensor op) so ScalarE runs Exp only
and never reloads its activation table.  Queries are sharded across the 8
cores; train_X and dual_coef are replicated.
"""

import numpy as np
import ml_dtypes

GAMMA = 1.0
N_QUERY, N_TRAIN, D = 8192, 8192, 512
N_CORES = 8
P = 128
QPC = N_QUERY // N_CORES  # 1024 queries per core
ND = D // P               # 4 contraction chunks
NT = N_TRAIN // P         # 64 train chunks
TGROUP = 8                # train chunks per resident tt DMA group
NTG = NT // TGROUP
QTILE = 512               # free dim of one sq-dist psum tile
NQC = QPC // QTILE        # 2 query chunks per core
NXC = QPC // P            # 8 query columns for x2 accumulation

_CACHE = {}


def _build_program(repeats=1):
    from contextlib import ExitStack

    import concourse.bass as bass
    import concourse.mybir as mybir
    import concourse.tile as tile
    from concourse import bacc

    f32 = mybir.dt.float32
    bf16 = mybir.dt.bfloat16
    AF = mybir.ActivationFunctionType
    MUL = mybir.AluOpType.mult

    nc = bacc.Bacc(
        "TRN2", target_bir_lowering=False, debug=False, num_devices=N_CORES
    )

    tt_d = nc.dram_tensor("tt_bf16", [D, N_TRAIN], bf16, kind="ExternalInput").ap()
    xt_d = nc.dram_tensor("xt_bf16", [D, QPC], bf16, kind="ExternalInput").ap()
    trf_d = nc.dram_tensor("train_f32", [N_TRAIN, D], f32, kind="ExternalInput").ap()
    xf_d = nc.dram_tensor("x_f32", [QPC, D], f32, kind="ExternalInput").ap()
    coef_d = nc.dram_tensor("coef_bf16", [P, NT], bf16, kind="ExternalInput").ap()
    out_d = nc.dram_tensor("out", [QPC], f32, kind="ExternalOutput").ap()
    x2r_d = nc.dram_tensor("x2_bounce", [QPC], f32).ap()  # internal scratch

    with tile.TileContext(nc) as tc, ExitStack() as ctx:
        res = ctx.enter_context(tc.tile_pool(name="res", bufs=1))
        ttp = ctx.enter_context(tc.tile_pool(name="ttp", bufs=1))
        stream = ctx.enter_context(tc.tile_pool(name="stream", bufs=4))
        exppool = ctx.enter_context(tc.tile_pool(name="expp", bufs=4))
        sqpool = ctx.enter_context(tc.tile_pool(name="psq", bufs=4, space="PSUM"))
        spool = ctx.enter_context(tc.tile_pool(name="pS", bufs=1, space="PSUM"))

        # ---- prologue: resident loads + x2 = rowwise ||X||^2 ----
        xt_sb = []
        for dc in range(ND):
            t = res.tile([P, QPC], bf16, tag=f"xt{dc}")
            nc.sync.dma_start(t[:], xt_d[dc * P : (dc + 1) * P, :])
            xt_sb.append(t)
        coef_sb = res.tile([P, NT], bf16, tag="coef")
        nc.sync.dma_start(coef_sb[:], coef_d[:])

        # x2 in column layout via DVE square+accumulate, then bounce through
        # DRAM to transpose into a single [1, QPC] row (hidden under main loop)
        x2_sb = res.tile([P, NXC], f32, tag="x2")
        for c in range(NXC):
            xtile = stream.tile([P, D], f32, tag="xf")
            nc.sync.dma_start(xtile[:], xf_d[c * P : (c + 1) * P, :])
            scr = stream.tile([P, D], bf16, tag="xscr")
            nc.vector.scalar_tensor_tensor(
                scr[:], xtile[:], 1.0, xtile[:], MUL, MUL,
                accum_out=x2_sb[:, c : c + 1],
            )
        nc.sync.dma_start(x2r_d.rearrange("(c p) -> p c", p=P), x2_sb[:])
        x2row = res.tile([1, QPC], f32, tag="x2row")
        nc.sync.dma_start(x2row[:], x2r_d.rearrange("(a q) -> a q", a=1))

        # ---- main loop over train chunks ----
        # S[qc] accumulates sum_t exp(...) * coef[t] as a [1, 512] psum row
        # per query chunk; each lives in its own psum bank so the long
        # accumulation groups never share a zero region.
        S_ps = [
            spool.tile([1, QTILE], f32, tag=f"S{qc}", name=f"S{qc}")
            for qc in range(NQC)
        ]
        for g in range(NTG):
            ttg = []
            for dc in range(ND):
                t = ttp.tile([P, TGROUP * P], bf16, tag=f"tt_{dc}_{g}")
                nc.sync.dma_start(
                    t[:],
                    tt_d[dc * P : (dc + 1) * P, g * TGROUP * P : (g + 1) * TGROUP * P],
                )
                ttg.append(t)
            for tl in range(TGROUP):
                ti = g * TGROUP + tl
                # y2n = -gamma * ||T[t]||^2 for this chunk (DVE, one op)
                trt = stream.tile([P, D], f32, tag="trf")
                nc.sync.dma_start(trt[:], trf_d[ti * P : (ti + 1) * P, :])
                scr2 = stream.tile([P, D], bf16, tag="trscr")
                y2nt = res.tile([P, 1], f32, tag=f"y2n_{ti}")
                nc.vector.scalar_tensor_tensor(
                    scr2[:], trt[:], -GAMMA, trt[:], MUL, MUL, accum_out=y2nt[:]
                )

                for qc in range(NQC):
                    ps = sqpool.tile([P, QTILE], f32, tag="sq")
                    for dc in range(ND):
                        nc.tensor.matmul(
                            ps[:],
                            ttg[dc][:, tl * P : (tl + 1) * P],
                            xt_sb[dc][:, qc * QTILE : (qc + 1) * QTILE],
                            start=(dc == 0),
                            stop=(dc == ND - 1),
                        )
                    et = exppool.tile([P, QTILE], bf16, tag="exp")
                    nc.scalar.activation(
                        et[:], ps[:], AF.Exp, bias=y2nt[:], scale=2.0 * GAMMA
                    )
                    nc.tensor.matmul(
                        S_ps[qc][:],
                        coef_sb[:, ti : ti + 1],
                        et[:],
                        start=(ti == 0),
                        stop=(ti == NT - 1),
                    )

        # ---- epilogue: out = exp(-g*x2) * S, all in row layout ----
        ex2 = res.tile([1, QPC], f32, tag="ex2")
        nc.scalar.activation(ex2[:], x2row[:], AF.Exp, scale=-GAMMA)
        outrow = res.tile([1, QPC], f32, tag="outrow")
        for qc in range(NQC):
            nc.vector.tensor_mul(
                outrow[:, qc * QTILE : (qc + 1) * QTILE],
                ex2[:, qc * QTILE : (qc + 1) * QTILE],
                S_ps[qc][:],
            )
        nc.sync.dma_start(out_d.rearrange("(a q) -> a q", a=1), outrow[:])

    nc.compile()
    return nc


def _get_program():
    if "nc" not in _CACHE:
        _CACHE["nc"] = _build_program()
    return _CACHE["nc"]


def make_in_maps(X, train_X, dual_coef):
    bf = ml_dtypes.bfloat16
    ttb = np.ascontiguousarray(train_X.T).astype(bf)
    coefb = np.ascontiguousarray(dual_coef.reshape(NT, P).T).astype(bf)
    XT = np.ascontiguousarray(X.T)
    in_maps = []
    for c in range(N_CORES):
        in_maps.append(
            {
                "tt_bf16": ttb,
                "xt_bf16": np.ascontiguousarray(XT[:, c * QPC : (c + 1) * QPC]).astype(
                    bf
                ),
                "train_f32": train_X,
                "x_f32": np.ascontiguousarray(X[c * QPC : (c + 1) * QPC]),
                "coef_bf16": coefb,
            }
        )
    return in_maps


def _get_callable():
    """Cached (fn, in_names, out_names, out_avals, zero_outs, mesh) for the
    sharded 8-core NEFF execution."""
    if "call" in _CACHE:
        return _CACHE["call"]

    import jax
    from jax.sharding import Mesh, PartitionSpec
    from jax.experimental.shard_map import shard_map

    import concourse.mybir as mybir
    from concourse import bass2jax
    from concourse.bass2jax import install_neuronx_cc_hook

    install_neuronx_cc_hook()
    nc = _get_program()

    partition_name = (
        nc.partition_id_tensor.name if nc.partition_id_tensor else None
    )
    in_names, out_names, out_avals, zero_outs = [], [], [], []
    for alloc in nc.m.functions[0].allocations:
        if not isinstance(alloc, mybir.MemoryLocationSet):
            continue
        if alloc.kind not in ("ExternalInput", "ExternalOutput"):
            continue
        name = alloc.memorylocations[0].name
        if alloc.kind == "ExternalInput":
            if name != partition_name:
                in_names.append(name)
        else:
            out_names.append(name)
            shape = tuple(alloc.tensor_shape)
            dtype = mybir.dt.np(alloc.dtype)
            out_avals.append(jax.core.ShapedArray(shape, dtype))
            zero_outs.append(np.zeros(shape, dtype))
    all_in_names = in_names + out_names
    if partition_name is not None:
        all_in_names = all_in_names + [partition_name]

    def _body(*args):
        operands = list(args)
        if partition_name is not None:
            operands.append(bass2jax.partition_id_tensor())
        outs = bass2jax._bass_exec_p.bind(
            *operands,
            out_avals=tuple(out_avals),
            in_names=tuple(all_in_names),
            out_names=tuple(out_names),
            lowering_input_output_aliases=(),
            sim_require_finite=True,
            sim_require_nnan=True,
            nc=nc,
        )
        return tuple(outs)

    devices = jax.devices()[:N_CORES]
    mesh = Mesh(np.asarray(devices), ("core",))
    n_all = len(in_names) + len(out_names)
    fn = jax.jit(
        shard_map(
            _body,
            mesh=mesh,
            in_specs=(PartitionSpec("core"),) * n_all,
            out_specs=(PartitionSpec("core"),) * len(out_names),
            check_rep=False,
        ),
        keep_unused=True,
    )
    _CACHE["call"] = (fn, in_names, out_names, out_avals, zero_outs, mesh)
    return _CACHE["call"]


def concat_inputs(in_maps):
    fn, in_names, out_names, out_avals, zero_outs, mesh = _get_callable()
    concat_in = [
        np.concatenate([np.asarray(m[name]) for m in in_maps], axis=0)
        for name in in_names
    ]
    concat_zeros = [
        np.zeros((N_CORES * z.shape[0], *z.shape[1:]), z.dtype) for z in zero_outs
    ]
    return concat_in + concat_zeros


def kernel(X, train_X, dual_coef):
    X = np.asarray(X, dtype=np.float32)
    train_X = np.asarray(train_X, dtype=np.float32)
    dual_coef = np.asarray(dual_coef, dtype=np.float32)

    fn, in_names, out_names, out_avals, zero_outs, mesh = _get_callable()
    in_maps = make_in_maps(X, train_X, dual_coef)
    args = concat_inputs(in_maps)
    outs = fn(*args)
    out = np.asarray(outs[0]).reshape(-1)
    return out.astype(np.float32)



# revision 2
# speedup vs baseline: 2.0636x; 2.0636x over previous
"""RBF kernel ridge regression inference on 8 Trainium2 NeuronCores.

out[q] = sum_t exp(-gamma * ||X[q] - T[t]||^2) * coef[t]

Factored as sum_t exp(2g*dot[q,t] - g*x2[q]) * (exp(-g*y2[t]) * coef[t]) so
each engine does what it is best at:

- TensorE: the (1024 x 512 x 8192) per-core GEMM in fp8 DoubleRow mode
  (virtual K=256 per matmul -> 256 matmuls instead of 512, ~1.4x faster).
  Output layout [q_part, t_free] so no TensorE matvec is needed at all.
- ScalarE: one Exp over each 4-bank PSUM group ([128, 2048] per ACTIVATE,
  bias = -g*x2[q] per partition) -> 32 ACTIVATEs instead of 128.
- VectorE: S[q] += sum_t et[q,t] * w[t] via scalar_tensor_tensor with
  free-axis accumulate (w replicated across partitions, bf16 2x mode).

Queries are sharded across the 8 cores; train side is replicated.  Host
precomputes the tiny O(N*d) prep: transposes, fp8/bf16 casts, row norms and
w[t] = exp(-g*y2[t])*coef[t] (0.05% of FLOPs; the 99.95% GEMM+exp+reduce run
on device).
"""

import numpy as np
import ml_dtypes

GAMMA = 1.0
N_QUERY, N_TRAIN, D = 8192, 8192, 512
N_CORES = 8
P = 128
QPC = N_QUERY // N_CORES  # 1024 queries per core
KS = D // P               # 4 contraction subtiles (d = ks*128 + p)
NQC = QPC // P            # 8 query chunks of 128
GT = 2048                 # train columns per psum group (4 banks)
NG = N_TRAIN // GT        # 4 groups
ST = 512                  # train cols per matmul (one psum bank)
NS = GT // ST             # 4 subtiles per group

_CACHE = {}


def _build_program():
    from contextlib import ExitStack

    import concourse.bass as bass
    import concourse.mybir as mybir
    import concourse.tile as tile
    from concourse import bacc

    f32 = mybir.dt.float32
    bf16 = mybir.dt.bfloat16
    f8 = mybir.dt.float8e4
    AF = mybir.ActivationFunctionType
    MUL = mybir.AluOpType.mult
    DR = mybir.MatmulPerfMode.DoubleRow

    nc = bacc.Bacc(
        "TRN2", target_bir_lowering=False, debug=False, num_devices=N_CORES
    )

    tt_d = nc.dram_tensor("tt_fp8", [D, N_TRAIN], f8, kind="ExternalInput").ap()
    x_d = nc.dram_tensor("x_fp8", [D, QPC], f8, kind="ExternalInput").ap()
    wb_d = nc.dram_tensor("wb_bf16", [P, N_TRAIN], bf16, kind="ExternalInput").ap()
    x2_d = nc.dram_tensor("x2n_f32", [P, NQC], f32, kind="ExternalInput").ap()
    out_d = nc.dram_tensor("out", [QPC], f32, kind="ExternalOutput").ap()

    with tile.TileContext(nc) as tc, ExitStack() as ctx:
        res = ctx.enter_context(tc.tile_pool(name="res", bufs=1))
        etp = ctx.enter_context(tc.tile_pool(name="etp", bufs=3))
        scrp = ctx.enter_context(tc.tile_pool(name="scrp", bufs=3))
        psq = ctx.enter_context(tc.tile_pool(name="psq", bufs=2, space="PSUM"))

        x_sb = res.tile([P, KS, QPC], f8, tag="x")
        x2_sb = res.tile([P, NQC], f32, tag="x2")
        wb_sb = res.tile([P, N_TRAIN], bf16, tag="wb")
        tt_sb = res.tile([P, KS, N_TRAIN], f8, tag="tt")
        Scol = res.tile([P, NQC * NG], f32, tag="Scol")
        out_sb = res.tile([P, NQC], f32, tag="out")

        # small resident loads first, then train tiles group-by-group so the
        # first matmuls start after ~1MB of DMA instead of the full 6.7MB
        for k in range(KS):
            nc.sync.dma_start(
                x_sb[:, k : k + 1, :],
                x_d[k * P : (k + 1) * P, :].rearrange("p (a q) -> p a q", a=1),
            )
        nc.sync.dma_start(x2_sb[:], x2_d[:])
        for g in range(NG):
            for k in range(KS):
                nc.sync.dma_start(
                    tt_sb[:, k : k + 1, g * GT : (g + 1) * GT],
                    tt_d[k * P : (k + 1) * P, g * GT : (g + 1) * GT].rearrange(
                        "p (a t) -> p a t", a=1
                    ),
                )
            nc.sync.dma_start(wb_sb[:, g * GT : (g + 1) * GT], wb_d[:, g * GT : (g + 1) * GT])

        for c in range(NQC):
            for g in range(NG):
                ps = psq.tile([P, GT], f32, tag="ps")
                for j in range(KS // 2):
                    for s in range(NS):
                        nc.tensor.matmul(
                            ps[:, s * ST : (s + 1) * ST],
                            x_sb[:, 2 * j : 2 * j + 2, c * P : (c + 1) * P],
                            tt_sb[:, 2 * j : 2 * j + 2, g * GT + s * ST : g * GT + (s + 1) * ST],
                            start=(j == 0),
                            stop=(j == KS // 2 - 1),
                            perf_mode=DR,
                        )
                et = etp.tile([P, GT], bf16, tag="et")
                nc.scalar.activation(
                    et[:], ps[:], AF.Exp, bias=x2_sb[:, c : c + 1], scale=2.0 * GAMMA
                )
                scr = scrp.tile([P, GT], bf16, tag="scr")
                nc.vector.scalar_tensor_tensor(
                    scr[:],
                    et[:],
                    1.0,
                    wb_sb[:, g * GT : (g + 1) * GT],
                    MUL,
                    MUL,
                    accum_out=Scol[:, c * NG + g : c * NG + g + 1],
                )
            nc.vector.tensor_reduce(
                out_sb[:, c : c + 1],
                Scol[:, c * NG : (c + 1) * NG],
                axis=mybir.AxisListType.X,
                op=mybir.AluOpType.add,
            )
        nc.sync.dma_start(out_d.rearrange("(c p) -> p c", p=P), out_sb[:])

    nc.compile()
    return nc


def _get_program():
    if "nc" not in _CACHE:
        _CACHE["nc"] = _build_program()
    return _CACHE["nc"]


def make_in_maps(X, train_X, dual_coef):
    bf = ml_dtypes.bfloat16
    f8 = ml_dtypes.float8_e4m3

    X = np.asarray(X, dtype=np.float32)
    train_X = np.asarray(train_X, dtype=np.float32)
    dual_coef = np.asarray(dual_coef, dtype=np.float32)

    ttq = np.ascontiguousarray(train_X.T).astype(f8)          # [D, N_TRAIN]
    y2 = np.einsum("td,td->t", train_X, train_X)              # [N_TRAIN]
    w = (np.exp(-GAMMA * y2) * dual_coef).astype(bf)          # [N_TRAIN]
    wb = np.ascontiguousarray(np.broadcast_to(w[None, :], (P, N_TRAIN)))
    x2 = np.einsum("qd,qd->q", X, X)                          # [N_QUERY]
    XT = np.ascontiguousarray(X.T)                            # [D, N_QUERY]

    in_maps = []
    for c in range(N_CORES):
        xs = np.ascontiguousarray(XT[:, c * QPC : (c + 1) * QPC]).astype(f8)
        x2c = np.ascontiguousarray(
            (-GAMMA * x2[c * QPC : (c + 1) * QPC]).reshape(NQC, P).T.astype(np.float32)
        )
        in_maps.append(
            {
                "tt_fp8": ttq,
                "x_fp8": xs,
                "wb_bf16": wb,
                "x2n_f32": x2c,
            }
        )
    return in_maps


def _get_callable():
    """Cached (fn, in_names, out_names, out_avals, zero_outs, mesh) for the
    sharded 8-core NEFF execution."""
    if "call" in _CACHE:
        return _CACHE["call"]

    import jax
    from jax.sharding import Mesh, PartitionSpec
    from jax.experimental.shard_map import shard_map

    import concourse.mybir as mybir
    from concourse import bass2jax
    from concourse.bass2jax import install_neuronx_cc_hook

    install_neuronx_cc_hook()
    nc = _get_program()

    partition_name = (
        nc.partition_id_tensor.name if nc.partition_id_tensor else None
    )
    in_names, out_names, out_avals, zero_outs = [], [], [], []
    for alloc in nc.m.functions[0].allocations:
        if not isinstance(alloc, mybir.MemoryLocationSet):
            continue
        if alloc.kind not in ("ExternalInput", "ExternalOutput"):
            continue
        name = alloc.memorylocations[0].name
        if alloc.kind == "ExternalInput":
            if name != partition_name:
                in_names.append(name)
        else:
            out_names.append(name)
            shape = tuple(alloc.tensor_shape)
            dtype = mybir.dt.np(alloc.dtype)
            out_avals.append(jax.core.ShapedArray(shape, dtype))
            zero_outs.append(np.zeros(shape, dtype))
    all_in_names = in_names + out_names
    if partition_name is not None:
        all_in_names = all_in_names + [partition_name]

    def _body(*args):
        operands = list(args)
        if partition_name is not None:
            operands.append(bass2jax.partition_id_tensor())
        outs = bass2jax._bass_exec_p.bind(
            *operands,
            out_avals=tuple(out_avals),
            in_names=tuple(all_in_names),
            out_names=tuple(out_names),
            lowering_input_output_aliases=(),
            sim_require_finite=True,
            sim_require_nnan=True,
            nc=nc,
        )
        return tuple(outs)

    devices = jax.devices()[:N_CORES]
    mesh = Mesh(np.asarray(devices), ("core",))
    n_all = len(in_names) + len(out_names)
    fn = jax.jit(
        shard_map(
            _body,
            mesh=mesh,
            in_specs=(PartitionSpec("core"),) * n_all,
            out_specs=(PartitionSpec("core"),) * len(out_names),
            check_rep=False,
        ),
        keep_unused=True,
    )
    _CACHE["call"] = (fn, in_names, out_names, out_avals, zero_outs, mesh)
    return _CACHE["call"]


def concat_inputs(in_maps):
    fn, in_names, out_names, out_avals, zero_outs, mesh = _get_callable()
    concat_in = [
        np.concatenate([np.asarray(m[name]) for m in in_maps], axis=0)
        for name in in_names
    ]
    concat_zeros = [
        np.zeros((N_CORES * z.shape[0], *z.shape[1:]), z.dtype) for z in zero_outs
    ]
    return concat_in + concat_zeros


def kernel(X, train_X, dual_coef):
    X = np.asarray(X, dtype=np.float32)
    train_X = np.asarray(train_X, dtype=np.float32)
    dual_coef = np.asarray(dual_coef, dtype=np.float32)

    fn, in_names, out_names, out_avals, zero_outs, mesh = _get_callable()
    in_maps = make_in_maps(X, train_X, dual_coef)
    args = concat_inputs(in_maps)
    outs = fn(*args)
    out = np.asarray(outs[0]).reshape(-1)
    return out.astype(np.float32)


# revision 12
# speedup vs baseline: 2.3004x; 1.1148x over previous
"""RBF kernel ridge regression inference on 8 Trainium2 NeuronCores.

out[q] = sum_t exp(-gamma * ||X[q] - T[t]||^2) * coef[t]

Factored as sum_t exp(2g*dot[q,t] - g*x2[q] - g*y2[t] + ln|coef[t]|) * sgn[t]:
the whole per-element weighting lives INSIDE the exp argument, so the
reduction over t needs no elementwise multiply at all.

- TensorE: fp8 DoubleRow GEMM in [q_part, t_free] layout (256 MMs instead of
  512), plus 4 tile-position-packed K=1 "bias" matmuls per psum tile that add
  the t-varying r[t] = ln|w[t]| - B row into the dot (outer product with a
  constant-8 stationary, moving r/8 in fp8; they run concurrently in distinct
  32-row strips, ~1 extra MM of cost per tile).
- ScalarE: one Exp per 4-bank psum group ([128, 2048], bias = -g*x2[q] + B
  per partition).  For sign-pure tiles (host sorts train points so negative
  coefs come first) the ACT's accum_out produces the tile's weighted sum for
  free.
- VectorE: for the remaining tiles, S partial = sum_t et'[q,t] * sgn[t] via
  scalar_tensor_tensor with a +-1 sign row (bf16).  The ScalarE/DVE split is
  chosen to balance both engines at ~64us.

Queries are sharded across the 8 cores; train side is replicated.  Host
precomputes the tiny O(N*d) prep: permutation, transposes, fp8/bf16 casts,
row norms, ln|w| folding (0.05% of FLOPs; the GEMM+exp+reduce run on device).
"""

import numpy as np
import ml_dtypes

GAMMA = 1.0
N_QUERY, N_TRAIN, D = 8192, 8192, 512
N_CORES = 8
P = 128
QPC = N_QUERY // N_CORES  # 1024 queries per core
KS = D // P               # 4 contraction subtiles (d = ks*128 + p)
NQC = QPC // P            # 8 query chunks of 128
GT = 2048                 # train columns per psum group (4 banks)
NG = N_TRAIN // GT        # 4 groups
ST = 512                  # train cols per matmul (one psum bank)
NS = GT // ST             # 4 subtiles per group
SCC = (0, 2, 3, 5, 7)     # q-chunks whose g=0 tile reduces on ScalarE

_CACHE = {}


def _build_program():
    from contextlib import ExitStack

    import concourse.bass as bass
    import concourse.mybir as mybir
    import concourse.tile as tile
    from concourse import bacc

    f32 = mybir.dt.float32
    bf16 = mybir.dt.bfloat16
    f8 = mybir.dt.float8e4
    AF = mybir.ActivationFunctionType
    MUL = mybir.AluOpType.mult
    DR = mybir.MatmulPerfMode.DoubleRow

    nc = bacc.Bacc(
        "TRN2", target_bir_lowering=False, debug=False, num_devices=N_CORES
    )

    tt_d = nc.dram_tensor("tt_fp8", [D, N_TRAIN], f8, kind="ExternalInput").ap()
    x_d = nc.dram_tensor("x_fp8", [D, QPC], f8, kind="ExternalInput").ap()
    sgn_d = nc.dram_tensor("sgn_bf16", [P, N_TRAIN], bf16, kind="ExternalInput").ap()
    r8_d = nc.dram_tensor("r8_fp8", [P, N_TRAIN], f8, kind="ExternalInput").ap()
    x2_d = nc.dram_tensor("x2n_f32", [P, NQC], f32, kind="ExternalInput").ap()
    out_d = nc.dram_tensor("out", [QPC], f32, kind="ExternalOutput").ap()

    with tile.TileContext(nc) as tc, ExitStack() as ctx:
        res = ctx.enter_context(tc.tile_pool(name="res", bufs=1))
        etp = ctx.enter_context(tc.tile_pool(name="etp", bufs=3))
        scrp = ctx.enter_context(tc.tile_pool(name="scrp", bufs=3))
        psq = ctx.enter_context(tc.tile_pool(name="psq", bufs=2, space="PSUM"))

        x_sb = res.tile([P, KS, QPC], f8, tag="x")
        x2_sb = res.tile([P, NQC], f32, tag="x2")
        sgn_sb = res.tile([P, N_TRAIN], bf16, tag="sgn")
        r8_sb = res.tile([P, N_TRAIN], f8, tag="r8")
        tt_sb = res.tile([P, KS, N_TRAIN], f8, tag="tt")
        Scol = res.tile([P, NQC * NG], f32, tag="Scol")
        Acol = res.tile([P, NQC], f32, tag="Acol")
        out_sb = res.tile([P, NQC], f32, tag="out")
        warm_sb = res.tile([P, 2, 128], f8, tag="warm")
        eights = res.tile([P, P], f8, tag="eights")
        ghost = res.tile([P, GT], bf16, tag="ghost")

        # loads split across the two hwdge queues (Sync + ScalarE) so the
        # first tiles' operands all land before the warmup matmuls finish
        nc.scalar.dma_start(x2_sb[:], x2_d[:])
        nc.scalar.dma_start(
            x_sb[:, 0:2, :], x_d[0 : 2 * P, :].rearrange("(k p) q -> p k q", k=2)
        )
        nc.scalar.dma_start(r8_sb[:, 0:GT], r8_d[:, 0:GT])
        nc.scalar.dma_start(
            x_sb[:, 2:4, :], x_d[2 * P :, :].rearrange("(k p) q -> p k q", k=2)
        )
        nc.scalar.dma_start(r8_sb[:, GT:], r8_d[:, GT:])
        nc.sync.dma_start(
            tt_sb[:, 0:2, 0:GT],
            tt_d[0 : 2 * P, 0:GT].rearrange("(k p) t -> p k t", k=2),
        )
        nc.sync.dma_start(
            tt_sb[:, 2:4, 0:GT],
            tt_d[2 * P :, 0:GT].rearrange("(k p) t -> p k t", k=2),
        )
        nc.sync.dma_start(sgn_sb[:, 0 : 2 * GT], sgn_d[:, 0 : 2 * GT])
        nc.sync.dma_start(
            tt_sb[:, :, GT : 2 * GT],
            tt_d[:, GT : 2 * GT].rearrange("(k p) t -> p k t", k=KS),
        )
        for g in range(2, NG):
            nc.sync.dma_start(
                tt_sb[:, :, g * GT : (g + 1) * GT],
                tt_d[:, g * GT : (g + 1) * GT].rearrange("(k p) t -> p k t", k=KS),
            )
        nc.sync.dma_start(sgn_sb[:, 2 * GT :], sgn_d[:, 2 * GT :])

        nc.vector.memset(warm_sb[:], 0)
        nc.vector.memset(eights[:], 8.0)
        nc.vector.memset(Scol[:], 0.0)
        # preload the exp table while DMAs stream (first real ACT would
        # otherwise pay the ~2.7us ACT_TABLE_LOAD on the critical path)
        warm_act = res.tile([P, 1], bf16, tag="wact")
        nc.scalar.activation(warm_act[:], warm_sb[:, 0, 0:1], AF.Exp, scale=1.0)

        # HAM warmup: keep the PE busy while the first train tiles stream in,
        # so the clock gate is at 8/8 when the real matmuls start.  Results
        # land in the first psum tile's banks and are discarded by the real
        # accumulation groups' start=True.
        ps0 = psq.tile([P, GT], f32, tag="ps")
        for r in range(36):
            nc.tensor.matmul(
                ps0[:, 0:128],
                warm_sb[:],
                warm_sb[:],
                start=True,
                stop=True,
                perf_mode=DR,
                skip_group_check=True,
            )

        first = True
        for g in range(NG):
            for c in range(NQC):
                ps = ps0 if first else psq.tile([P, GT], f32, tag="ps")
                first = False
                for s in range(NS):
                    for j in range(KS // 2):
                        nc.tensor.matmul(
                            ps[:, s * ST : (s + 1) * ST],
                            x_sb[:, 2 * j : 2 * j + 2, c * P : (c + 1) * P],
                            tt_sb[:, 2 * j : 2 * j + 2, g * GT + s * ST : g * GT + (s + 1) * ST],
                            start=(j == 0),
                            stop=False,
                            perf_mode=DR,
                        )
                # r[t] row added via 4 concurrently-packed K=1 matmuls
                # (distinct 32-row strips, distinct psum banks):
                # ps[:, bank s] += 8 * (r[t]/8)
                for s in range(NS):
                    nc.tensor.matmul(
                        ps[:, s * ST : (s + 1) * ST],
                        eights[32 * s : 32 * s + 1, :],
                        r8_sb[32 * s : 32 * s + 1, g * GT + s * ST : g * GT + (s + 1) * ST],
                        start=False,
                        stop=True,
                        tile_position=(32 * s, 0),
                    )
                if g == 0 and c in SCC:
                    # sign-pure tile: ScalarE reduces it for free via accum
                    nc.scalar.activation(
                        ghost[:],
                        ps[:],
                        AF.Exp,
                        bias=x2_sb[:, c : c + 1],
                        scale=2.0 * GAMMA,
                        accum_out=Acol[:, c : c + 1],
                    )
                else:
                    et = etp.tile([P, GT], bf16, tag="et")
                    nc.scalar.activation(
                        et[:], ps[:], AF.Exp, bias=x2_sb[:, c : c + 1], scale=2.0 * GAMMA
                    )
                    scr = scrp.tile([P, GT], bf16, tag="scr")
                    nc.vector.scalar_tensor_tensor(
                        scr[:],
                        et[:],
                        1.0,
                        sgn_sb[:, g * GT : (g + 1) * GT],
                        MUL,
                        MUL,
                        accum_out=Scol[:, c * NG + g : c * NG + g + 1],
                    )
        for c in range(NQC):
            nc.vector.tensor_reduce(
                out_sb[:, c : c + 1],
                Scol[:, c * NG : (c + 1) * NG],
                axis=mybir.AxisListType.X,
                op=mybir.AluOpType.add,
            )
            if c in SCC:
                # g=0 tile is all-negative-coef: subtract its ScalarE accum
                nc.vector.tensor_tensor(
                    out_sb[:, c : c + 1],
                    out_sb[:, c : c + 1],
                    Acol[:, c : c + 1],
                    mybir.AluOpType.subtract,
                )
        # p-major out layout: per-partition contiguous 32B runs instead of
        # 1024 scattered 4B descriptors; kernel() un-permutes on host
        nc.sync.dma_start(out_d.rearrange("(p c) -> p c", p=P), out_sb[:])

    nc.compile()
    return nc


def _get_program():
    if "nc" not in _CACHE:
        _CACHE["nc"] = _build_program()
    return _CACHE["nc"]


def make_in_maps(X, train_X, dual_coef):
    bf = ml_dtypes.bfloat16
    f8 = ml_dtypes.float8_e4m3

    X = np.asarray(X, dtype=np.float32)
    train_X = np.asarray(train_X, dtype=np.float32)
    dual_coef = np.asarray(dual_coef, dtype=np.float32)

    # flip so negative coefs are the majority (>= 4096 >= GT); the host
    # negates the final output back.  Then sort negatives first so the
    # g=0 tiles (first GT columns) are sign-pure for the ScalarE reduction.
    flip = (dual_coef < 0).sum() < N_TRAIN // 2
    coef = -dual_coef if flip else dual_coef
    perm = np.concatenate([np.where(coef < 0)[0], np.where(coef >= 0)[0]])
    coef = coef[perm]
    train_s = train_X[perm]

    ttq = np.ascontiguousarray(train_s.T).astype(f8)          # [D, N_TRAIN]
    y2 = np.einsum("td,td->t", train_s, train_s)              # [N_TRAIN]
    lnw = -GAMMA * y2 + np.log(np.maximum(np.abs(coef), 1e-30))
    B = float(np.mean(lnw))
    r8 = ((lnw - B) / 8.0).astype(f8)                         # fits e4m3
    r8b = np.ascontiguousarray(np.broadcast_to(r8[None, :], (P, N_TRAIN)))
    sgn = np.sign(coef).astype(bf)
    sgnb = np.ascontiguousarray(np.broadcast_to(sgn[None, :], (P, N_TRAIN)))
    x2 = np.einsum("qd,qd->q", X, X)                          # [N_QUERY]
    XT = np.ascontiguousarray(X.T)                            # [D, N_QUERY]

    in_maps = []
    for c in range(N_CORES):
        xs = np.ascontiguousarray(XT[:, c * QPC : (c + 1) * QPC]).astype(f8)
        x2c = np.ascontiguousarray(
            (-GAMMA * x2[c * QPC : (c + 1) * QPC] + B)
            .reshape(NQC, P)
            .T.astype(np.float32)
        )
        in_maps.append(
            {
                "tt_fp8": ttq,
                "x_fp8": xs,
                "sgn_bf16": sgnb,
                "r8_fp8": r8b,
                "x2n_f32": x2c,
            }
        )
    return in_maps, flip


def _get_callable():
    """Cached (fn, in_names, out_names, out_avals, zero_outs, mesh) for the
    sharded 8-core NEFF execution."""
    if "call" in _CACHE:
        return _CACHE["call"]

    import jax
    from jax.sharding import Mesh, PartitionSpec
    from jax.experimental.shard_map import shard_map

    import concourse.mybir as mybir
    from concourse import bass2jax
    from concourse.bass2jax import install_neuronx_cc_hook

    install_neuronx_cc_hook()
    nc = _get_program()

    partition_name = (
        nc.partition_id_tensor.name if nc.partition_id_tensor else None
    )
    in_names, out_names, out_avals, zero_outs = [], [], [], []
    for alloc in nc.m.functions[0].allocations:
        if not isinstance(alloc, mybir.MemoryLocationSet):
            continue
        if alloc.kind not in ("ExternalInput", "ExternalOutput"):
            continue
        name = alloc.memorylocations[0].name
        if alloc.kind == "ExternalInput":
            if name != partition_name:
                in_names.append(name)
        else:
            out_names.append(name)
            shape = tuple(alloc.tensor_shape)
            dtype = mybir.dt.np(alloc.dtype)
            out_avals.append(jax.core.ShapedArray(shape, dtype))
            zero_outs.append(np.zeros(shape, dtype))
    all_in_names = in_names + out_names
    if partition_name is not None:
        all_in_names = all_in_names + [partition_name]

    def _body(*args):
        operands = list(args)
        if partition_name is not None:
            operands.append(bass2jax.partition_id_tensor())
        outs = bass2jax._bass_exec_p.bind(
            *operands,
            out_avals=tuple(out_avals),
            in_names=tuple(all_in_names),
            out_names=tuple(out_names),
            lowering_input_output_aliases=(),
            sim_require_finite=True,
            sim_require_nnan=True,
            nc=nc,
        )
        return tuple(outs)

    devices = jax.devices()[:N_CORES]
    mesh = Mesh(np.asarray(devices), ("core",))
    n_all = len(in_names) + len(out_names)
    fn = jax.jit(
        shard_map(
            _body,
            mesh=mesh,
            in_specs=(PartitionSpec("core"),) * n_all,
            out_specs=(PartitionSpec("core"),) * len(out_names),
            check_rep=False,
        ),
        keep_unused=True,
    )
    _CACHE["call"] = (fn, in_names, out_names, out_avals, zero_outs, mesh)
    return _CACHE["call"]


def concat_inputs(in_maps):
    fn, in_names, out_names, out_avals, zero_outs, mesh = _get_callable()
    concat_in = [
        np.concatenate([np.asarray(m[name]) for m in in_maps], axis=0)
        for name in in_names
    ]
    concat_zeros = [
        np.zeros((N_CORES * z.shape[0], *z.shape[1:]), z.dtype) for z in zero_outs
    ]
    return concat_in + concat_zeros


def kernel(X, train_X, dual_coef):
    X = np.asarray(X, dtype=np.float32)
    train_X = np.asarray(train_X, dtype=np.float32)
    dual_coef = np.asarray(dual_coef, dtype=np.float32)

    fn, in_names, out_names, out_avals, zero_outs, mesh = _get_callable()
    in_maps, flip = make_in_maps(X, train_X, dual_coef)
    args = concat_inputs(in_maps)
    outs = fn(*args)
    out = np.asarray(outs[0]).reshape(N_CORES, P, NQC)
    # device wrote p-major ([p, c] with q = c*128 + p); un-permute per core
    out = out.transpose(0, 2, 1).reshape(-1)
    if flip:
        out = -out
    return np.ascontiguousarray(out).astype(np.float32)


# revision 16
# speedup vs baseline: 2.4123x; 1.0487x over previous
"""RBF kernel ridge regression inference on 8 Trainium2 NeuronCores.

out[q] = sum_t exp(-gamma * ||X[q] - T[t]||^2) * coef[t]

Factored as sum_t exp(2g*dot[q,t] - g*x2[q] - g*y2[t] + ln|coef[t]|) * sgn[t]:
the whole per-element weighting lives INSIDE the exp argument, so the
reduction over t needs no elementwise multiply at all.

- TensorE: fp8 DoubleRow GEMM in [q_part, t_free] layout (256 MMs instead of
  512), plus 4 tile-position-packed K=1 "bias" matmuls per psum tile that add
  the t-varying r[t] = ln|w[t]| - B row into the dot (outer product with a
  constant-8 stationary, moving r/8 in fp8; they run concurrently in distinct
  32-row strips, ~1 extra MM of cost per tile).
- ScalarE: one Exp per 4-bank psum group ([128, 2048], bias = -g*x2[q] + B
  per partition).  For sign-pure tiles (host sorts train points so negative
  coefs come first) the ACT's accum_out produces the tile's weighted sum for
  free.
- VectorE: for the remaining tiles, S partial = sum_t et'[q,t] * sgn[t] via
  scalar_tensor_tensor with a +-1 sign row (bf16).  The ScalarE/DVE split is
  chosen to balance both engines at ~64us.

Queries are sharded across the 8 cores; train side is replicated.  Host
precomputes the tiny O(N*d) prep: permutation, transposes, fp8/bf16 casts,
row norms, ln|w| folding (0.05% of FLOPs; the GEMM+exp+reduce run on device).
"""

import numpy as np
import ml_dtypes

GAMMA = 1.0
N_QUERY, N_TRAIN, D = 8192, 8192, 512
N_CORES = 8
P = 128
QPC = N_QUERY // N_CORES  # 1024 queries per core
KS = D // P               # 4 contraction subtiles (d = ks*128 + p)
NQC = QPC // P            # 8 query chunks of 128
GT = 2048                 # train columns per psum group (4 banks)
NG = N_TRAIN // GT        # 4 groups
ST = 512                  # train cols per matmul (one psum bank)
NS = GT // ST             # 4 subtiles per group
SCC = (0, 2, 3, 5, 7)     # q-chunks whose g=0 tile reduces on ScalarE

_CACHE = {}


def _build_program():
    from contextlib import ExitStack

    import concourse.bass as bass
    import concourse.mybir as mybir
    import concourse.tile as tile
    from concourse import bacc

    f32 = mybir.dt.float32
    bf16 = mybir.dt.bfloat16
    f8 = mybir.dt.float8e4
    AF = mybir.ActivationFunctionType
    MUL = mybir.AluOpType.mult
    DR = mybir.MatmulPerfMode.DoubleRow

    nc = bacc.Bacc(
        "TRN2", target_bir_lowering=False, debug=False, num_devices=N_CORES
    )

    tt_d = nc.dram_tensor("tt_fp8", [D, N_TRAIN], f8, kind="ExternalInput").ap()
    x_d = nc.dram_tensor("x_fp8", [D, QPC], f8, kind="ExternalInput").ap()
    sgn_d = nc.dram_tensor("sgn_bf16", [P, N_TRAIN], bf16, kind="ExternalInput").ap()
    r8_d = nc.dram_tensor("r8_fp8", [1, N_TRAIN], f8, kind="ExternalInput").ap()
    x2_d = nc.dram_tensor("x2n_f32", [P, NQC], f32, kind="ExternalInput").ap()
    out_d = nc.dram_tensor("out", [QPC], f32, kind="ExternalOutput").ap()

    with tile.TileContext(nc) as tc, ExitStack() as ctx:
        res = ctx.enter_context(tc.tile_pool(name="res", bufs=1))
        etp = ctx.enter_context(tc.tile_pool(name="etp", bufs=3))
        scrp = ctx.enter_context(tc.tile_pool(name="scrp", bufs=3))
        psq = ctx.enter_context(tc.tile_pool(name="psq", bufs=2, space="PSUM"))

        x_sb = res.tile([P, KS, QPC], f8, tag="x")
        x2_sb = res.tile([P, NQC], f32, tag="x2")
        sgn_sb = res.tile([P, N_TRAIN], bf16, tag="sgn")
        r8_sb = res.tile([P, N_TRAIN], f8, tag="r8")
        tt_sb = res.tile([P, KS, N_TRAIN], f8, tag="tt")
        Scol = res.tile([P, NQC * NG], f32, tag="Scol")
        Acol = res.tile([P, NQC], f32, tag="Acol")
        out_sb = res.tile([P, NQC], f32, tag="out")
        warm_sb = res.tile([P, 2, 128], f8, tag="warm")
        eights = res.tile([P, P], f8, tag="eights")
        ghost = res.tile([P, GT], bf16, tag="ghost")

        # loads split across the two hwdge queues (Sync + ScalarE); the
        # non-critical streams get wait_until floors so the first tiles'
        # operands (~1.6MB) get the full DMA bandwidth instead of sharing
        # it with 5MB of later-needed data
        nc.scalar.dma_start(x2_sb[:], x2_d[:])
        nc.scalar.dma_start(
            x_sb[:, 0:2, :], x_d[0 : 2 * P, :].rearrange("(k p) q -> p k q", k=2)
        )
        nc.scalar.dma_start(
            x_sb[:, 2:4, :], x_d[2 * P :, :].rearrange("(k p) q -> p k q", k=2)
        )
        for b in range(NS):
            nc.scalar.dma_start(r8_sb[32 * b : 32 * b + 1, :], r8_d[:])
        nc.sync.dma_start(
            tt_sb[:, 0:2, 0:GT],
            tt_d[0 : 2 * P, 0:GT].rearrange("(k p) t -> p k t", k=2),
        )
        nc.sync.dma_start(
            tt_sb[:, 2:4, 0:GT],
            tt_d[2 * P :, 0:GT].rearrange("(k p) t -> p k t", k=2),
        )
        with tc.tile_wait_until(0.006):
            nc.sync.dma_start(sgn_sb[:, 0:GT], sgn_d[:, 0:GT])
        with tc.tile_wait_until(0.010):
            nc.sync.dma_start(
                tt_sb[:, :, GT : 2 * GT],
                tt_d[:, GT : 2 * GT].rearrange("(k p) t -> p k t", k=KS),
            )
        with tc.tile_wait_until(0.014):
            nc.sync.dma_start(sgn_sb[:, GT : 2 * GT], sgn_d[:, GT : 2 * GT])
        with tc.tile_wait_until(0.020):
            nc.sync.dma_start(
                tt_sb[:, :, 2 * GT : 3 * GT],
                tt_d[:, 2 * GT : 3 * GT].rearrange("(k p) t -> p k t", k=KS),
            )
        with tc.tile_wait_until(0.028):
            nc.sync.dma_start(
                tt_sb[:, :, 3 * GT :],
                tt_d[:, 3 * GT :].rearrange("(k p) t -> p k t", k=KS),
            )
            nc.sync.dma_start(sgn_sb[:, 2 * GT :], sgn_d[:, 2 * GT :])

        nc.vector.memset(warm_sb[:], 0)
        nc.vector.memset(eights[:], 8.0)
        nc.vector.memset(Scol[:], 0.0)
        # preload the exp table while DMAs stream (first real ACT would
        # otherwise pay the ~2.7us ACT_TABLE_LOAD on the critical path)
        warm_act = res.tile([P, 1], bf16, tag="wact")
        nc.scalar.activation(warm_act[:], warm_sb[:, 0, 0:1], AF.Exp, scale=1.0)

        # HAM warmup: keep the PE busy while the first train tiles stream in,
        # so the clock gate is at 8/8 when the real matmuls start.  Results
        # land in the first psum tile's banks and are discarded by the real
        # accumulation groups' start=True.
        ps0 = psq.tile([P, GT], f32, tag="ps")
        for r in range(36):
            nc.tensor.matmul(
                ps0[:, 0:128],
                warm_sb[:],
                warm_sb[:],
                start=True,
                stop=True,
                perf_mode=DR,
                skip_group_check=True,
            )

        first = True
        for g in range(NG):
            for c in range(NQC):
                ps = ps0 if first else psq.tile([P, GT], f32, tag="ps")
                first = False
                for s in range(NS):
                    for j in range(KS // 2):
                        nc.tensor.matmul(
                            ps[:, s * ST : (s + 1) * ST],
                            x_sb[:, 2 * j : 2 * j + 2, c * P : (c + 1) * P],
                            tt_sb[:, 2 * j : 2 * j + 2, g * GT + s * ST : g * GT + (s + 1) * ST],
                            start=(j == 0),
                            stop=False,
                            perf_mode=DR,
                        )
                # r[t] row added via 4 concurrently-packed K=1 matmuls
                # (distinct 32-row strips, distinct psum banks):
                # ps[:, bank s] += 8 * (r[t]/8)
                for s in range(NS):
                    nc.tensor.matmul(
                        ps[:, s * ST : (s + 1) * ST],
                        eights[32 * s : 32 * s + 1, :],
                        r8_sb[32 * s : 32 * s + 1, g * GT + s * ST : g * GT + (s + 1) * ST],
                        start=False,
                        stop=True,
                        tile_position=(32 * s, 0),
                    )
                if g == 0 and c in SCC:
                    # sign-pure tile: ScalarE reduces it for free via accum
                    nc.scalar.activation(
                        ghost[:],
                        ps[:],
                        AF.Exp,
                        bias=x2_sb[:, c : c + 1],
                        scale=2.0 * GAMMA,
                        accum_out=Acol[:, c : c + 1],
                    )
                else:
                    et = etp.tile([P, GT], bf16, tag="et")
                    nc.scalar.activation(
                        et[:], ps[:], AF.Exp, bias=x2_sb[:, c : c + 1], scale=2.0 * GAMMA
                    )
                    scr = scrp.tile([P, GT], bf16, tag="scr")
                    nc.vector.scalar_tensor_tensor(
                        scr[:],
                        et[:],
                        1.0,
                        sgn_sb[:, g * GT : (g + 1) * GT],
                        MUL,
                        MUL,
                        accum_out=Scol[:, c * NG + g : c * NG + g + 1],
                    )
        for c in range(NQC):
            nc.vector.tensor_reduce(
                out_sb[:, c : c + 1],
                Scol[:, c * NG : (c + 1) * NG],
                axis=mybir.AxisListType.X,
                op=mybir.AluOpType.add,
            )
            if c in SCC:
                # g=0 tile is all-negative-coef: subtract its ScalarE accum
                nc.vector.tensor_tensor(
                    out_sb[:, c : c + 1],
                    out_sb[:, c : c + 1],
                    Acol[:, c : c + 1],
                    mybir.AluOpType.subtract,
                )
        # p-major out layout: per-partition contiguous 32B runs instead of
        # 1024 scattered 4B descriptors; kernel() un-permutes on host
        nc.sync.dma_start(out_d.rearrange("(p c) -> p c", p=P), out_sb[:])

    nc.compile()
    return nc


def _get_program():
    if "nc" not in _CACHE:
        _CACHE["nc"] = _build_program()
    return _CACHE["nc"]


def make_in_maps(X, train_X, dual_coef):
    bf = ml_dtypes.bfloat16
    f8 = ml_dtypes.float8_e4m3

    X = np.asarray(X, dtype=np.float32)
    train_X = np.asarray(train_X, dtype=np.float32)
    dual_coef = np.asarray(dual_coef, dtype=np.float32)

    # flip so negative coefs are the majority (>= 4096 >= GT); the host
    # negates the final output back.  Then sort negatives first so the
    # g=0 tiles (first GT columns) are sign-pure for the ScalarE reduction.
    flip = (dual_coef < 0).sum() < N_TRAIN // 2
    coef = -dual_coef if flip else dual_coef
    perm = np.concatenate([np.where(coef < 0)[0], np.where(coef >= 0)[0]])
    coef = coef[perm]
    train_s = train_X[perm]

    ttq = np.ascontiguousarray(train_s.T).astype(f8)          # [D, N_TRAIN]
    y2 = np.einsum("td,td->t", train_s, train_s)              # [N_TRAIN]
    lnw = -GAMMA * y2 + np.log(np.maximum(np.abs(coef), 1e-30))
    B = float(np.mean(lnw))
    r8 = ((lnw - B) / 8.0).astype(f8).reshape(1, N_TRAIN)     # fits e4m3
    sgn = np.sign(coef).astype(bf)
    sgnb = np.ascontiguousarray(np.broadcast_to(sgn[None, :], (P, N_TRAIN)))
    x2 = np.einsum("qd,qd->q", X, X)                          # [N_QUERY]
    XT = np.ascontiguousarray(X.T)                            # [D, N_QUERY]

    in_maps = []
    for c in range(N_CORES):
        xs = np.ascontiguousarray(XT[:, c * QPC : (c + 1) * QPC]).astype(f8)
        x2c = np.ascontiguousarray(
            (-GAMMA * x2[c * QPC : (c + 1) * QPC] + B)
            .reshape(NQC, P)
            .T.astype(np.float32)
        )
        in_maps.append(
            {
                "tt_fp8": ttq,
                "x_fp8": xs,
                "sgn_bf16": sgnb,
                "r8_fp8": r8,
                "x2n_f32": x2c,
            }
        )
    return in_maps, flip


def _get_callable():
    """Cached (fn, in_names, out_names, out_avals, zero_outs, mesh) for the
    sharded 8-core NEFF execution."""
    if "call" in _CACHE:
        return _CACHE["call"]

    import jax
    from jax.sharding import Mesh, PartitionSpec
    from jax.experimental.shard_map import shard_map

    import concourse.mybir as mybir
    from concourse import bass2jax
    from concourse.bass2jax import install_neuronx_cc_hook

    install_neuronx_cc_hook()
    nc = _get_program()

    partition_name = (
        nc.partition_id_tensor.name if nc.partition_id_tensor else None
    )
    in_names, out_names, out_avals, zero_outs = [], [], [], []
    for alloc in nc.m.functions[0].allocations:
        if not isinstance(alloc, mybir.MemoryLocationSet):
            continue
        if alloc.kind not in ("ExternalInput", "ExternalOutput"):
            continue
        name = alloc.memorylocations[0].name
        if alloc.kind == "ExternalInput":
            if name != partition_name:
                in_names.append(name)
        else:
            out_names.append(name)
            shape = tuple(alloc.tensor_shape)
            dtype = mybir.dt.np(alloc.dtype)
            out_avals.append(jax.core.ShapedArray(shape, dtype))
            zero_outs.append(np.zeros(shape, dtype))
    all_in_names = in_names + out_names
    if partition_name is not None:
        all_in_names = all_in_names + [partition_name]

    def _body(*args):
        operands = list(args)
        if partition_name is not None:
            operands.append(bass2jax.partition_id_tensor())
        outs = bass2jax._bass_exec_p.bind(
            *operands,
            out_avals=tuple(out_avals),
            in_names=tuple(all_in_names),
            out_names=tuple(out_names),
            lowering_input_output_aliases=(),
            sim_require_finite=True,
            sim_require_nnan=True,
            nc=nc,
        )
        return tuple(outs)

    devices = jax.devices()[:N_CORES]
    mesh = Mesh(np.asarray(devices), ("core",))
    n_all = len(in_names) + len(out_names)
    fn = jax.jit(
        shard_map(
            _body,
            mesh=mesh,
            in_specs=(PartitionSpec("core"),) * n_all,
            out_specs=(PartitionSpec("core"),) * len(out_names),
            check_rep=False,
        ),
        keep_unused=True,
    )
    _CACHE["call"] = (fn, in_names, out_names, out_avals, zero_outs, mesh)
    return _CACHE["call"]


def concat_inputs(in_maps):
    fn, in_names, out_names, out_avals, zero_outs, mesh = _get_callable()
    concat_in = [
        np.concatenate([np.asarray(m[name]) for m in in_maps], axis=0)
        for name in in_names
    ]
    concat_zeros = [
        np.zeros((N_CORES * z.shape[0], *z.shape[1:]), z.dtype) for z in zero_outs
    ]
    return concat_in + concat_zeros


def kernel(X, train_X, dual_coef):
    X = np.asarray(X, dtype=np.float32)
    train_X = np.asarray(train_X, dtype=np.float32)
    dual_coef = np.asarray(dual_coef, dtype=np.float32)

    fn, in_names, out_names, out_avals, zero_outs, mesh = _get_callable()
    in_maps, flip = make_in_maps(X, train_X, dual_coef)
    args = concat_inputs(in_maps)
    outs = fn(*args)
    out = np.asarray(outs[0]).reshape(N_CORES, P, NQC)
    # device wrote p-major ([p, c] with q = c*128 + p); un-permute per core
    out = out.transpose(0, 2, 1).reshape(-1)
    if flip:
        out = -out
    return np.ascontiguousarray(out).astype(np.float32)


# revision 23
# speedup vs baseline: 2.6131x; 1.0832x over previous
"""RBF kernel ridge regression inference on 8 Trainium2 NeuronCores.

out[q] = sum_t exp(-gamma * ||X[q] - T[t]||^2) * coef[t]

Factored as sum_t exp(2g*dot[q,t] - g*x2[q] - g*y2[t] + ln|coef[t]|) * sgn[t]:
the whole per-element weighting lives INSIDE the exp argument, so the
reduction over t needs no elementwise multiply at all.

- TensorE: fp8 DoubleRow GEMM in [q_part, t_free] layout (256 MMs instead of
  512), plus 4 tile-position-packed K=1 "bias" matmuls per psum tile that add
  the t-varying r[t] = ln|w[t]| - B row into the dot (outer product with a
  constant-8 stationary, moving r/8 in fp8; they run concurrently in distinct
  32-row strips, ~1 extra MM of cost per tile).
- ScalarE: one Exp per 4-bank psum group ([128, 2048], bias = -g*x2[q] + B
  per partition).  For sign-pure tiles (host sorts train points so negative
  coefs come first) the ACT's accum_out produces the tile's weighted sum for
  free.
- VectorE: for the remaining tiles, S partial = sum_t et'[q,t] * sgn[t] via
  scalar_tensor_tensor with a +-1 sign row (bf16).  The ScalarE/DVE split is
  chosen to balance both engines at ~64us.

Queries are sharded across the 8 cores; train side is replicated.  Host
precomputes the tiny O(N*d) prep: permutation, transposes, fp8/bf16 casts,
row norms, ln|w| folding (0.05% of FLOPs; the GEMM+exp+reduce run on device).
"""

import numpy as np
import ml_dtypes

GAMMA = 1.0
N_QUERY, N_TRAIN, D = 8192, 8192, 512
N_CORES = 8
P = 128
QPC = N_QUERY // N_CORES  # 1024 queries per core
KS = D // P               # 4 contraction subtiles (d = ks*128 + p)
NQC = QPC // P            # 8 query chunks of 128
GT = 2048                 # train columns per psum group (4 banks)
NG = N_TRAIN // GT        # 4 groups
ST = 512                  # train cols per matmul (one psum bank)
NS = GT // ST             # 4 subtiles per group
# (g, c) tiles whose reduction runs on ScalarE via ACT accum_out.  Host
# sign-sorts train points (negatives first, majority negative after flip),
# so g=0/g=1 tiles are all-negative and g=3 tiles all-positive; g=2 holds
# the sign boundary and must reduce on DVE (sign-row multiply).  The g=3
# entries cover the last-processed tiles so the kernel doesn't end on a
# trailing DVE op.
SCC_NEG = ((0, 2), (0, 6), (1, 1), (1, 5))
SCC_POS = ((3, 4), (3, 5), (3, 6), (3, 7))
SCC = SCC_NEG + SCC_POS

_CACHE = {}


def _build_program():
    from contextlib import ExitStack

    import concourse.bass as bass
    import concourse.mybir as mybir
    import concourse.tile as tile
    from concourse import bacc

    f32 = mybir.dt.float32
    bf16 = mybir.dt.bfloat16
    f8 = mybir.dt.float8e4
    AF = mybir.ActivationFunctionType
    MUL = mybir.AluOpType.mult
    DR = mybir.MatmulPerfMode.DoubleRow

    nc = bacc.Bacc(
        "TRN2", target_bir_lowering=False, debug=False, num_devices=N_CORES
    )

    tt_d = nc.dram_tensor("tt_fp8", [D, N_TRAIN], f8, kind="ExternalInput").ap()
    x_d = nc.dram_tensor("x_fp8", [D, QPC], f8, kind="ExternalInput").ap()
    wb_d = nc.dram_tensor("wb_bf16", [P, N_TRAIN], bf16, kind="ExternalInput").ap()
    r8_d = nc.dram_tensor("r8_fp8", [1, N_TRAIN], f8, kind="ExternalInput").ap()
    x2_d = nc.dram_tensor("x2n_f32", [P, 2 * NQC], f32, kind="ExternalInput").ap()
    out_d = nc.dram_tensor("out", [QPC], f32, kind="ExternalOutput").ap()

    with tile.TileContext(nc) as tc, ExitStack() as ctx:
        res = ctx.enter_context(tc.tile_pool(name="res", bufs=1))
        etp = ctx.enter_context(tc.tile_pool(name="etp", bufs=3))
        scrp = ctx.enter_context(tc.tile_pool(name="scrp", bufs=3))
        psq = ctx.enter_context(tc.tile_pool(name="psq", bufs=2, space="PSUM"))

        x_sb = res.tile([P, KS, QPC], f8, tag="x")
        x2_sb = res.tile([P, 2 * NQC], f32, tag="x2")
        wb_sb = res.tile([P, N_TRAIN], bf16, tag="wb")
        r8_sb = res.tile([P, N_TRAIN], f8, tag="r8")
        tt_sb = res.tile([P, KS, N_TRAIN], f8, tag="tt")
        Scol = res.tile([P, NQC * NG], f32, tag="Scol")
        Acol = res.tile([P, max(1, len(SCC))], f32, tag="Acol")
        out_sb = res.tile([P, NQC], f32, tag="out")
        warm_sb = res.tile([P, 2, 128], f8, tag="warm")
        eights = res.tile([P, P], f8, tag="eights")
        ghost = res.tile([P, GT], bf16, tag="ghost")

        # loads split across the two hwdge queues (Sync + ScalarE); the
        # non-critical streams get wait_until floors so the first tiles'
        # operands (~1.6MB) get the full DMA bandwidth instead of sharing
        # it with 5MB of later-needed data
        nc.scalar.dma_start(x2_sb[:], x2_d[:])
        nc.scalar.dma_start(
            x_sb[:, 0:2, :], x_d[0 : 2 * P, :].rearrange("(k p) q -> p k q", k=2)
        )
        nc.scalar.dma_start(
            x_sb[:, 2:4, :], x_d[2 * P :, :].rearrange("(k p) q -> p k q", k=2)
        )
        for b in range(NS):
            nc.scalar.dma_start(r8_sb[32 * b : 32 * b + 1, :], r8_d[:])
        nc.sync.dma_start(
            tt_sb[:, 0:2, 0:GT],
            tt_d[0 : 2 * P, 0:GT].rearrange("(k p) t -> p k t", k=2),
        )
        nc.sync.dma_start(
            tt_sb[:, 2:4, 0:GT],
            tt_d[2 * P :, 0:GT].rearrange("(k p) t -> p k t", k=2),
        )
        with tc.tile_wait_until(0.006):
            nc.sync.dma_start(wb_sb[:, 0:GT], wb_d[:, 0:GT])
        with tc.tile_wait_until(0.010):
            nc.sync.dma_start(
                tt_sb[:, :, GT : 2 * GT],
                tt_d[:, GT : 2 * GT].rearrange("(k p) t -> p k t", k=KS),
            )
        with tc.tile_wait_until(0.014):
            nc.sync.dma_start(wb_sb[:, GT : 2 * GT], wb_d[:, GT : 2 * GT])
        with tc.tile_wait_until(0.020):
            nc.sync.dma_start(
                tt_sb[:, :, 2 * GT : 3 * GT],
                tt_d[:, 2 * GT : 3 * GT].rearrange("(k p) t -> p k t", k=KS),
            )
        with tc.tile_wait_until(0.028):
            nc.sync.dma_start(
                tt_sb[:, :, 3 * GT :],
                tt_d[:, 3 * GT :].rearrange("(k p) t -> p k t", k=KS),
            )
            nc.sync.dma_start(wb_sb[:, 2 * GT :], wb_d[:, 2 * GT :])

        nc.vector.memset(warm_sb[:], 0)
        nc.vector.memset(eights[:], 8.0)
        nc.vector.memset(Scol[:], 0.0)
        # preload the exp table while DMAs stream (first real ACT would
        # otherwise pay the ~2.7us ACT_TABLE_LOAD on the critical path)
        warm_act = res.tile([P, 1], bf16, tag="wact")
        nc.scalar.activation(warm_act[:], warm_sb[:, 0, 0:1], AF.Exp, scale=1.0)

        # HAM warmup: keep the PE busy while the first train tiles stream in,
        # so the clock gate is at 8/8 when the real matmuls start.  Results
        # land in the first psum tile's banks and are discarded by the real
        # accumulation groups' start=True.
        ps0 = psq.tile([P, GT], f32, tag="ps")
        for r in range(36):
            nc.tensor.matmul(
                ps0[:, 0:128],
                warm_sb[:],
                warm_sb[:],
                start=True,
                stop=True,
                perf_mode=DR,
                skip_group_check=True,
            )

        first = True
        for g in range(NG):
            for c in range(NQC):
                ps = ps0 if first else psq.tile([P, GT], f32, tag="ps")
                first = False
                sc_tile = (g, c) in SCC
                for s in range(NS):
                    for j in range(KS // 2):
                        nc.tensor.matmul(
                            ps[:, s * ST : (s + 1) * ST],
                            x_sb[:, 2 * j : 2 * j + 2, c * P : (c + 1) * P],
                            tt_sb[:, 2 * j : 2 * j + 2, g * GT + s * ST : g * GT + (s + 1) * ST],
                            start=(j == 0),
                            stop=(j == KS // 2 - 1) and not sc_tile,
                            perf_mode=DR,
                        )
                if sc_tile:
                    # ScalarE-reduced tile: add the t-varying ln|w[t]| - B row
                    # via 4 concurrently-packed K=1 matmuls (distinct 32-row
                    # strips, distinct psum banks): ps[:, bank s] += 8*(r/8)
                    for s in range(NS):
                        nc.tensor.matmul(
                            ps[:, s * ST : (s + 1) * ST],
                            eights[32 * s : 32 * s + 1, :],
                            r8_sb[32 * s : 32 * s + 1, g * GT + s * ST : g * GT + (s + 1) * ST],
                            start=False,
                            stop=True,
                            tile_position=(32 * s, 0),
                        )
                    # sign-pure tile: ScalarE reduces it for free via accum
                    nc.scalar.activation(
                        ghost[:],
                        ps[:],
                        AF.Exp,
                        bias=x2_sb[:, NQC + c : NQC + c + 1],
                        scale=2.0 * GAMMA,
                        accum_out=Acol[:, SCC.index((g, c)) : SCC.index((g, c)) + 1],
                    )
                else:
                    et = etp.tile([P, GT], bf16, tag="et")
                    nc.scalar.activation(
                        et[:], ps[:], AF.Exp, bias=x2_sb[:, c : c + 1], scale=2.0 * GAMMA
                    )
                    scr = scrp.tile([P, GT], bf16, tag="scr")
                    nc.vector.scalar_tensor_tensor(
                        scr[:],
                        et[:],
                        1.0,
                        wb_sb[:, g * GT : (g + 1) * GT],
                        MUL,
                        MUL,
                        accum_out=Scol[:, c * NG + g : c * NG + g + 1],
                    )
        for c in range(NQC):
            nc.vector.tensor_reduce(
                out_sb[:, c : c + 1],
                Scol[:, c * NG : (c + 1) * NG],
                axis=mybir.AxisListType.X,
                op=mybir.AluOpType.add,
            )
            for (g2, c2) in SCC_NEG:
                if c2 == c:
                    # all-negative-coef tile: subtract its ScalarE accum
                    nc.vector.tensor_tensor(
                        out_sb[:, c : c + 1],
                        out_sb[:, c : c + 1],
                        Acol[:, SCC.index((g2, c2)) : SCC.index((g2, c2)) + 1],
                        mybir.AluOpType.subtract,
                    )
            for (g2, c2) in SCC_POS:
                if c2 == c:
                    nc.vector.tensor_tensor(
                        out_sb[:, c : c + 1],
                        out_sb[:, c : c + 1],
                        Acol[:, SCC.index((g2, c2)) : SCC.index((g2, c2)) + 1],
                        mybir.AluOpType.add,
                    )
        # p-major out layout: per-partition contiguous 32B runs instead of
        # 1024 scattered 4B descriptors; kernel() un-permutes on host
        nc.sync.dma_start(out_d.rearrange("(p c) -> p c", p=P), out_sb[:])

    nc.compile()
    return nc


def _get_program():
    if "nc" not in _CACHE:
        _CACHE["nc"] = _build_program()
    return _CACHE["nc"]


def make_in_maps(X, train_X, dual_coef):
    bf = ml_dtypes.bfloat16
    f8 = ml_dtypes.float8_e4m3

    X = np.asarray(X, dtype=np.float32)
    train_X = np.asarray(train_X, dtype=np.float32)
    dual_coef = np.asarray(dual_coef, dtype=np.float32)

    # flip so negative coefs are the majority (>= 4096 >= GT); the host
    # negates the final output back.  Then sort negatives first so the
    # g=0 tiles (first GT columns) are sign-pure for the ScalarE reduction.
    flip = (dual_coef < 0).sum() < N_TRAIN // 2
    coef = -dual_coef if flip else dual_coef
    perm = np.concatenate([np.where(coef < 0)[0], np.where(coef >= 0)[0]])
    coef = coef[perm]
    train_s = train_X[perm]

    ttq = np.ascontiguousarray(train_s.T).astype(f8)          # [D, N_TRAIN]
    y2 = np.einsum("td,td->t", train_s, train_s)              # [N_TRAIN]
    lnw = -GAMMA * y2 + np.log(np.maximum(np.abs(coef), 1e-30))
    B = float(np.mean(lnw))
    # ACT applies scale=2g to the whole psum (dot + r-row), so pre-divide
    # r by 2g; /8 pairs with the constant-8 stationary of the bias matmuls
    r8 = ((lnw - B) / (2.0 * GAMMA) / 8.0).astype(f8).reshape(1, N_TRAIN)
    w = (np.exp(-GAMMA * y2) * coef).astype(bf)               # DVE-tile weights
    wbb = np.ascontiguousarray(np.broadcast_to(w[None, :], (P, N_TRAIN)))
    x2 = np.einsum("qd,qd->q", X, X)                          # [N_QUERY]
    XT = np.ascontiguousarray(X.T)                            # [D, N_QUERY]

    in_maps = []
    for c in range(N_CORES):
        xs = np.ascontiguousarray(XT[:, c * QPC : (c + 1) * QPC]).astype(f8)
        x2n = -GAMMA * x2[c * QPC : (c + 1) * QPC]
        x2c = np.ascontiguousarray(
            np.concatenate(
                [x2n.reshape(NQC, P).T, (x2n + B).reshape(NQC, P).T], axis=1
            ).astype(np.float32)
        )
        in_maps.append(
            {
                "tt_fp8": ttq,
                "x_fp8": xs,
                "wb_bf16": wbb,
                "r8_fp8": r8,
                "x2n_f32": x2c,
            }
        )
    return in_maps, flip


def _get_callable():
    """Cached (fn, in_names, out_names, out_avals, zero_outs, mesh) for the
    sharded 8-core NEFF execution."""
    if "call" in _CACHE:
        return _CACHE["call"]

    import jax
    from jax.sharding import Mesh, PartitionSpec
    from jax.experimental.shard_map import shard_map

    import concourse.mybir as mybir
    from concourse import bass2jax
    from concourse.bass2jax import install_neuronx_cc_hook

    install_neuronx_cc_hook()
    nc = _get_program()

    partition_name = (
        nc.partition_id_tensor.name if nc.partition_id_tensor else None
    )
    in_names, out_names, out_avals, zero_outs = [], [], [], []
    for alloc in nc.m.functions[0].allocations:
        if not isinstance(alloc, mybir.MemoryLocationSet):
            continue
        if alloc.kind not in ("ExternalInput", "ExternalOutput"):
            continue
        name = alloc.memorylocations[0].name
        if alloc.kind == "ExternalInput":
            if name != partition_name:
                in_names.append(name)
        else:
            out_names.append(name)
            shape = tuple(alloc.tensor_shape)
            dtype = mybir.dt.np(alloc.dtype)
            out_avals.append(jax.core.ShapedArray(shape, dtype))
            zero_outs.append(np.zeros(shape, dtype))
    all_in_names = in_names + out_names
    if partition_name is not None:
        all_in_names = all_in_names + [partition_name]

    def _body(*args):
        operands = list(args)
        if partition_name is not None:
            operands.append(bass2jax.partition_id_tensor())
        outs = bass2jax._bass_exec_p.bind(
            *operands,
            out_avals=tuple(out_avals),
            in_names=tuple(all_in_names),
            out_names=tuple(out_names),
            lowering_input_output_aliases=(),
            sim_require_finite=True,
            sim_require_nnan=True,
            nc=nc,
        )
        return tuple(outs)

    devices = jax.devices()[:N_CORES]
    mesh = Mesh(np.asarray(devices), ("core",))
    n_all = len(in_names) + len(out_names)
    fn = jax.jit(
        shard_map(
            _body,
            mesh=mesh,
            in_specs=(PartitionSpec("core"),) * n_all,
            out_specs=(PartitionSpec("core"),) * len(out_names),
            check_rep=False,
        ),
        keep_unused=True,
    )
    _CACHE["call"] = (fn, in_names, out_names, out_avals, zero_outs, mesh)
    return _CACHE["call"]


def concat_inputs(in_maps):
    fn, in_names, out_names, out_avals, zero_outs, mesh = _get_callable()
    concat_in = [
        np.concatenate([np.asarray(m[name]) for m in in_maps], axis=0)
        for name in in_names
    ]
    concat_zeros = [
        np.zeros((N_CORES * z.shape[0], *z.shape[1:]), z.dtype) for z in zero_outs
    ]
    return concat_in + concat_zeros


def kernel(X, train_X, dual_coef):
    X = np.asarray(X, dtype=np.float32)
    train_X = np.asarray(train_X, dtype=np.float32)
    dual_coef = np.asarray(dual_coef, dtype=np.float32)

    fn, in_names, out_names, out_avals, zero_outs, mesh = _get_callable()
    in_maps, flip = make_in_maps(X, train_X, dual_coef)
    args = concat_inputs(in_maps)
    outs = fn(*args)
    out = np.asarray(outs[0]).reshape(N_CORES, P, NQC)
    # device wrote p-major ([p, c] with q = c*128 + p); un-permute per core
    out = out.transpose(0, 2, 1).reshape(-1)
    if flip:
        out = -out
    return np.ascontiguousarray(out).astype(np.float32)


# revision 25
# speedup vs baseline: 2.6575x; 1.0170x over previous
"""RBF kernel ridge regression inference on 8 Trainium2 NeuronCores.

out[q] = sum_t exp(-gamma * ||X[q] - T[t]||^2) * coef[t],
with w[t] = exp(-g*y2[t]) * coef[t] so out[q] = sum_t exp(2g*dot - g*x2) * w.

All three compute engines are balanced at ~2.1us per [128q, 2048t] tile:

- TensorE: fp8 DoubleRow GEMM in [q_part, t_free] layout (256 MMs instead of
  512; warm MMs issue every ~216ns when consecutive MMs hit the same psum
  bank, hence the s-outer/j-inner order).
- ScalarE: one Exp per 4-bank psum group ([128, 2048] ACTIVATE, per-partition
  bias -g*x2[q]).
- Reduction over t, split to whichever engine has slack:
  * 24 tiles on VectorE: S += sum_t et[q,t] * w[t] via scalar_tensor_tensor
    free-axis accumulate (w as a broadcast bf16 row).
  * 8 sign-pure tiles on ScalarE for free via ACTIVATE's accum_out: the host
    sign-sorts train points (negatives first, flipped so negatives are the
    majority), and |w[t]| is folded INTO the exp argument as a t-varying row
    r[t] = (ln|w[t]| - B)/2g added to the dot by 4 tile-position-packed K=1
    matmuls (concurrent in distinct 32-row strips, ~0.6us per tile); the
    per-partition ACT bias carries -g*x2[q] + B.  accum then yields
    +-sum_t et*|w| directly; signs are applied in the final combine.

Startup hides the ~22us of replicated-input DMA: non-critical streams get
tile_wait_until floors so the first tile's operands get full bandwidth, and
36 throwaway DoubleRow matmuls keep the PE busy so the HAM clock gate is at
8/8 when real work starts; the exp table is preloaded the same way.

Queries are sharded across the 8 cores; train side is replicated.  Host
precomputes the tiny O(N*d) prep: permutation, transposes, fp8/bf16 casts,
row norms, ln|w| folding (0.05% of FLOPs; the GEMM+exp+reduce run on device).
"""

import numpy as np
import ml_dtypes

GAMMA = 1.0
N_QUERY, N_TRAIN, D = 8192, 8192, 512
N_CORES = 8
P = 128
QPC = N_QUERY // N_CORES  # 1024 queries per core
KS = D // P               # 4 contraction subtiles (d = ks*128 + p)
NQC = QPC // P            # 8 query chunks of 128
GT = 2048                 # train columns per psum group (4 banks)
NG = N_TRAIN // GT        # 4 groups
ST = 512                  # train cols per matmul (one psum bank)
NS = GT // ST             # 4 subtiles per group
# (g, c) tiles whose reduction runs on ScalarE via ACT accum_out.  Host
# sign-sorts train points (negatives first, majority negative after flip),
# so g=0/g=1 tiles are all-negative and g=3 tiles all-positive; g=2 holds
# the sign boundary and must reduce on DVE (sign-row multiply).  The g=3
# entries cover the last-processed tiles so the kernel doesn't end on a
# trailing DVE op.
SCC_NEG = ((0, 2), (0, 6), (1, 1), (1, 5))
SCC_POS = ((3, 4), (3, 5), (3, 6), (3, 7))
SCC = SCC_NEG + SCC_POS

_CACHE = {}


def _build_program():
    from contextlib import ExitStack

    import concourse.bass as bass
    import concourse.mybir as mybir
    import concourse.tile as tile
    from concourse import bacc

    f32 = mybir.dt.float32
    bf16 = mybir.dt.bfloat16
    f8 = mybir.dt.float8e4
    AF = mybir.ActivationFunctionType
    MUL = mybir.AluOpType.mult
    DR = mybir.MatmulPerfMode.DoubleRow

    nc = bacc.Bacc(
        "TRN2", target_bir_lowering=False, debug=False, num_devices=N_CORES
    )

    tt_d = nc.dram_tensor("tt_fp8", [D, N_TRAIN], f8, kind="ExternalInput").ap()
    x_d = nc.dram_tensor("x_fp8", [D, QPC], f8, kind="ExternalInput").ap()
    wb_d = nc.dram_tensor("wb_bf16", [P, N_TRAIN], bf16, kind="ExternalInput").ap()
    r8_d = nc.dram_tensor("r8_fp8", [1, N_TRAIN], f8, kind="ExternalInput").ap()
    x2_d = nc.dram_tensor("x2n_f32", [P, 2 * NQC], f32, kind="ExternalInput").ap()
    out_d = nc.dram_tensor("out", [QPC], f32, kind="ExternalOutput").ap()

    with tile.TileContext(nc) as tc, ExitStack() as ctx:
        res = ctx.enter_context(tc.tile_pool(name="res", bufs=1))
        etp = ctx.enter_context(tc.tile_pool(name="etp", bufs=4))
        scrp = ctx.enter_context(tc.tile_pool(name="scrp", bufs=4))
        psq = ctx.enter_context(tc.tile_pool(name="psq", bufs=2, space="PSUM"))

        x_sb = res.tile([P, KS, QPC], f8, tag="x")
        x2_sb = res.tile([P, 2 * NQC], f32, tag="x2")
        wb_sb = res.tile([P, N_TRAIN], bf16, tag="wb")
        r8_sb = res.tile([P, N_TRAIN], f8, tag="r8")
        tt_sb = res.tile([P, KS, N_TRAIN], f8, tag="tt")
        Scol = res.tile([P, NQC * NG], f32, tag="Scol")
        Acol = res.tile([P, max(1, len(SCC))], f32, tag="Acol")
        out_sb = res.tile([P, NQC], f32, tag="out")
        warm_sb = res.tile([P, 2, 128], f8, tag="warm")
        eights = res.tile([P, P], f8, tag="eights")
        ghost = res.tile([P, GT], bf16, tag="ghost")

        # loads split across the two hwdge queues (Sync + ScalarE); the
        # non-critical streams get wait_until floors so the first tiles'
        # operands (~1.6MB) get the full DMA bandwidth instead of sharing
        # it with 5MB of later-needed data
        nc.scalar.dma_start(x2_sb[:], x2_d[:])
        nc.scalar.dma_start(
            x_sb[:, 0:2, :], x_d[0 : 2 * P, :].rearrange("(k p) q -> p k q", k=2)
        )
        nc.scalar.dma_start(
            x_sb[:, 2:4, :], x_d[2 * P :, :].rearrange("(k p) q -> p k q", k=2)
        )
        for b in range(NS):
            nc.scalar.dma_start(r8_sb[32 * b : 32 * b + 1, :], r8_d[:])
        nc.sync.dma_start(
            tt_sb[:, 0:2, 0:GT],
            tt_d[0 : 2 * P, 0:GT].rearrange("(k p) t -> p k t", k=2),
        )
        nc.sync.dma_start(
            tt_sb[:, 2:4, 0:GT],
            tt_d[2 * P :, 0:GT].rearrange("(k p) t -> p k t", k=2),
        )
        with tc.tile_wait_until(0.009):
            nc.sync.dma_start(wb_sb[:, 0:GT], wb_d[:, 0:GT])
        with tc.tile_wait_until(0.013):
            nc.sync.dma_start(
                tt_sb[:, :, GT : 2 * GT],
                tt_d[:, GT : 2 * GT].rearrange("(k p) t -> p k t", k=KS),
            )
        with tc.tile_wait_until(0.017):
            nc.sync.dma_start(wb_sb[:, GT : 2 * GT], wb_d[:, GT : 2 * GT])
        with tc.tile_wait_until(0.020):
            nc.sync.dma_start(
                tt_sb[:, :, 2 * GT : 3 * GT],
                tt_d[:, 2 * GT : 3 * GT].rearrange("(k p) t -> p k t", k=KS),
            )
        with tc.tile_wait_until(0.028):
            nc.sync.dma_start(
                tt_sb[:, :, 3 * GT :],
                tt_d[:, 3 * GT :].rearrange("(k p) t -> p k t", k=KS),
            )
            nc.sync.dma_start(wb_sb[:, 2 * GT :], wb_d[:, 2 * GT :])

        nc.vector.memset(warm_sb[:], 0)
        nc.vector.memset(eights[:], 8.0)
        nc.vector.memset(Scol[:], 0.0)
        # preload the exp table while DMAs stream (first real ACT would
        # otherwise pay the ~2.7us ACT_TABLE_LOAD on the critical path)
        warm_act = res.tile([P, 1], bf16, tag="wact")
        nc.scalar.activation(warm_act[:], warm_sb[:, 0, 0:1], AF.Exp, scale=1.0)

        # HAM warmup: keep the PE busy while the first train tiles stream in,
        # so the clock gate is at 8/8 when the real matmuls start.  Results
        # land in the first psum tile's banks and are discarded by the real
        # accumulation groups' start=True.
        ps0 = psq.tile([P, GT], f32, tag="ps")
        for r in range(36):
            nc.tensor.matmul(
                ps0[:, 0:128],
                warm_sb[:],
                warm_sb[:],
                start=True,
                stop=True,
                perf_mode=DR,
                skip_group_check=True,
            )

        first = True
        for g in range(NG):
            for c in range(NQC):
                ps = ps0 if first else psq.tile([P, GT], f32, tag="ps")
                first = False
                sc_tile = (g, c) in SCC
                for s in range(NS):
                    for j in range(KS // 2):
                        nc.tensor.matmul(
                            ps[:, s * ST : (s + 1) * ST],
                            x_sb[:, 2 * j : 2 * j + 2, c * P : (c + 1) * P],
                            tt_sb[:, 2 * j : 2 * j + 2, g * GT + s * ST : g * GT + (s + 1) * ST],
                            start=(j == 0),
                            stop=(j == KS // 2 - 1) and not sc_tile,
                            perf_mode=DR,
                        )
                if sc_tile:
                    # ScalarE-reduced tile: add the t-varying ln|w[t]| - B row
                    # via 4 concurrently-packed K=1 matmuls (distinct 32-row
                    # strips, distinct psum banks): ps[:, bank s] += 8*(r/8)
                    for s in range(NS):
                        nc.tensor.matmul(
                            ps[:, s * ST : (s + 1) * ST],
                            eights[32 * s : 32 * s + 1, :],
                            r8_sb[32 * s : 32 * s + 1, g * GT + s * ST : g * GT + (s + 1) * ST],
                            start=False,
                            stop=True,
                            tile_position=(32 * s, 0),
                        )
                    # sign-pure tile: ScalarE reduces it for free via accum
                    nc.scalar.activation(
                        ghost[:],
                        ps[:],
                        AF.Exp,
                        bias=x2_sb[:, NQC + c : NQC + c + 1],
                        scale=2.0 * GAMMA,
                        accum_out=Acol[:, SCC.index((g, c)) : SCC.index((g, c)) + 1],
                    )
                else:
                    et = etp.tile([P, GT], bf16, tag="et")
                    nc.scalar.activation(
                        et[:], ps[:], AF.Exp, bias=x2_sb[:, c : c + 1], scale=2.0 * GAMMA
                    )
                    scr = scrp.tile([P, GT], bf16, tag="scr")
                    nc.vector.scalar_tensor_tensor(
                        scr[:],
                        et[:],
                        1.0,
                        wb_sb[:, g * GT : (g + 1) * GT],
                        MUL,
                        MUL,
                        accum_out=Scol[:, c * NG + g : c * NG + g + 1],
                    )
        for c in range(NQC):
            nc.vector.tensor_reduce(
                out_sb[:, c : c + 1],
                Scol[:, c * NG : (c + 1) * NG],
                axis=mybir.AxisListType.X,
                op=mybir.AluOpType.add,
            )
            for (g2, c2) in SCC_NEG:
                if c2 == c:
                    # all-negative-coef tile: subtract its ScalarE accum
                    nc.vector.tensor_tensor(
                        out_sb[:, c : c + 1],
                        out_sb[:, c : c + 1],
                        Acol[:, SCC.index((g2, c2)) : SCC.index((g2, c2)) + 1],
                        mybir.AluOpType.subtract,
                    )
            for (g2, c2) in SCC_POS:
                if c2 == c:
                    nc.vector.tensor_tensor(
                        out_sb[:, c : c + 1],
                        out_sb[:, c : c + 1],
                        Acol[:, SCC.index((g2, c2)) : SCC.index((g2, c2)) + 1],
                        mybir.AluOpType.add,
                    )
        # p-major out layout: per-partition contiguous 32B runs instead of
        # 1024 scattered 4B descriptors; kernel() un-permutes on host
        nc.sync.dma_start(out_d.rearrange("(p c) -> p c", p=P), out_sb[:])

    nc.compile()
    return nc


def _get_program():
    if "nc" not in _CACHE:
        _CACHE["nc"] = _build_program()
    return _CACHE["nc"]


def make_in_maps(X, train_X, dual_coef):
    bf = ml_dtypes.bfloat16
    f8 = ml_dtypes.float8_e4m3

    X = np.asarray(X, dtype=np.float32)
    train_X = np.asarray(train_X, dtype=np.float32)
    dual_coef = np.asarray(dual_coef, dtype=np.float32)

    # flip so negative coefs are the majority (>= 4096 >= GT); the host
    # negates the final output back.  Then sort negatives first so the
    # g=0 tiles (first GT columns) are sign-pure for the ScalarE reduction.
    flip = (dual_coef < 0).sum() < N_TRAIN // 2
    coef = -dual_coef if flip else dual_coef
    perm = np.concatenate([np.where(coef < 0)[0], np.where(coef >= 0)[0]])
    coef = coef[perm]
    train_s = train_X[perm]

    ttq = np.ascontiguousarray(train_s.T).astype(f8)          # [D, N_TRAIN]
    y2 = np.einsum("td,td->t", train_s, train_s)              # [N_TRAIN]
    lnw = -GAMMA * y2 + np.log(np.maximum(np.abs(coef), 1e-30))
    B = float(np.mean(lnw))
    # ACT applies scale=2g to the whole psum (dot + r-row), so pre-divide
    # r by 2g; /8 pairs with the constant-8 stationary of the bias matmuls
    r8 = ((lnw - B) / (2.0 * GAMMA) / 8.0).astype(f8).reshape(1, N_TRAIN)
    w = (np.exp(-GAMMA * y2) * coef).astype(bf)               # DVE-tile weights
    wbb = np.ascontiguousarray(np.broadcast_to(w[None, :], (P, N_TRAIN)))
    x2 = np.einsum("qd,qd->q", X, X)                          # [N_QUERY]
    XT = np.ascontiguousarray(X.T)                            # [D, N_QUERY]

    in_maps = []
    for c in range(N_CORES):
        xs = np.ascontiguousarray(XT[:, c * QPC : (c + 1) * QPC]).astype(f8)
        x2n = -GAMMA * x2[c * QPC : (c + 1) * QPC]
        x2c = np.ascontiguousarray(
            np.concatenate(
                [x2n.reshape(NQC, P).T, (x2n + B).reshape(NQC, P).T], axis=1
            ).astype(np.float32)
        )
        in_maps.append(
            {
                "tt_fp8": ttq,
                "x_fp8": xs,
                "wb_bf16": wbb,
                "r8_fp8": r8,
                "x2n_f32": x2c,
            }
        )
    return in_maps, flip


def _get_callable():
    """Cached (fn, in_names, out_names, out_avals, zero_outs, mesh) for the
    sharded 8-core NEFF execution."""
    if "call" in _CACHE:
        return _CACHE["call"]

    import jax
    from jax.sharding import Mesh, PartitionSpec
    from jax.experimental.shard_map import shard_map

    import concourse.mybir as mybir
    from concourse import bass2jax
    from concourse.bass2jax import install_neuronx_cc_hook

    install_neuronx_cc_hook()
    nc = _get_program()

    partition_name = (
        nc.partition_id_tensor.name if nc.partition_id_tensor else None
    )
    in_names, out_names, out_avals, zero_outs = [], [], [], []
    for alloc in nc.m.functions[0].allocations:
        if not isinstance(alloc, mybir.MemoryLocationSet):
            continue
        if alloc.kind not in ("ExternalInput", "ExternalOutput"):
            continue
        name = alloc.memorylocations[0].name
        if alloc.kind == "ExternalInput":
            if name != partition_name:
                in_names.append(name)
        else:
            out_names.append(name)
            shape = tuple(alloc.tensor_shape)
            dtype = mybir.dt.np(alloc.dtype)
            out_avals.append(jax.core.ShapedArray(shape, dtype))
            zero_outs.append(np.zeros(shape, dtype))
    all_in_names = in_names + out_names
    if partition_name is not None:
        all_in_names = all_in_names + [partition_name]

    def _body(*args):
        operands = list(args)
        if partition_name is not None:
            operands.append(bass2jax.partition_id_tensor())
        outs = bass2jax._bass_exec_p.bind(
            *operands,
            out_avals=tuple(out_avals),
            in_names=tuple(all_in_names),
            out_names=tuple(out_names),
            lowering_input_output_aliases=(),
            sim_require_finite=True,
            sim_require_nnan=True,
            nc=nc,
        )
        return tuple(outs)

    devices = jax.devices()[:N_CORES]
    mesh = Mesh(np.asarray(devices), ("core",))
    n_all = len(in_names) + len(out_names)
    fn = jax.jit(
        shard_map(
            _body,
            mesh=mesh,
            in_specs=(PartitionSpec("core"),) * n_all,
            out_specs=(PartitionSpec("core"),) * len(out_names),
            check_rep=False,
        ),
        keep_unused=True,
    )
    _CACHE["call"] = (fn, in_names, out_names, out_avals, zero_outs, mesh)
    return _CACHE["call"]


def concat_inputs(in_maps):
    fn, in_names, out_names, out_avals, zero_outs, mesh = _get_callable()
    concat_in = [
        np.concatenate([np.asarray(m[name]) for m in in_maps], axis=0)
        for name in in_names
    ]
    concat_zeros = [
        np.zeros((N_CORES * z.shape[0], *z.shape[1:]), z.dtype) for z in zero_outs
    ]
    return concat_in + concat_zeros


def kernel(X, train_X, dual_coef):
    X = np.asarray(X, dtype=np.float32)
    train_X = np.asarray(train_X, dtype=np.float32)
    dual_coef = np.asarray(dual_coef, dtype=np.float32)

    fn, in_names, out_names, out_avals, zero_outs, mesh = _get_callable()
    in_maps, flip = make_in_maps(X, train_X, dual_coef)
    args = concat_inputs(in_maps)
    outs = fn(*args)
    out = np.asarray(outs[0]).reshape(N_CORES, P, NQC)
    # device wrote p-major ([p, c] with q = c*128 + p); un-permute per core
    out = out.transpose(0, 2, 1).reshape(-1)
    if flip:
        out = -out
    return np.ascontiguousarray(out).astype(np.float32)


# revision 26
# speedup vs baseline: 2.6993x; 1.0158x over previous
"""RBF kernel ridge regression inference on 8 Trainium2 NeuronCores.

out[q] = sum_t exp(-gamma * ||X[q] - T[t]||^2) * coef[t],
with w[t] = exp(-g*y2[t]) * coef[t] so out[q] = sum_t exp(2g*dot - g*x2) * w.

All three compute engines are balanced at ~2.1us per [128q, 2048t] tile:

- TensorE: fp8 DoubleRow GEMM in [q_part, t_free] layout (256 MMs instead of
  512; warm MMs issue every ~216ns when consecutive MMs hit the same psum
  bank, hence the s-outer/j-inner order).
- ScalarE: one Exp per 4-bank psum group ([128, 2048] ACTIVATE, per-partition
  bias -g*x2[q]).
- Reduction over t, split to whichever engine has slack:
  * 24 tiles on VectorE: S += sum_t et[q,t] * w[t] via scalar_tensor_tensor
    free-axis accumulate (w as a broadcast bf16 row).
  * 8 sign-pure tiles on ScalarE for free via ACTIVATE's accum_out: the host
    sign-sorts train points (negatives first, flipped so negatives are the
    majority), and |w[t]| is folded INTO the exp argument as a t-varying row
    r[t] = (ln|w[t]| - B)/2g added to the dot by 4 tile-position-packed K=1
    matmuls (concurrent in distinct 32-row strips, ~0.6us per tile); the
    per-partition ACT bias carries -g*x2[q] + B.  accum then yields
    +-sum_t et*|w| directly; signs are applied in the final combine.

Startup hides the ~22us of replicated-input DMA: non-critical streams get
tile_wait_until floors so the first tile's operands get full bandwidth, and
36 throwaway DoubleRow matmuls keep the PE busy so the HAM clock gate is at
8/8 when real work starts; the exp table is preloaded the same way.

Queries are sharded across the 8 cores; train side is replicated.  Host
precomputes the tiny O(N*d) prep: permutation, transposes, fp8/bf16 casts,
row norms, ln|w| folding (0.05% of FLOPs; the GEMM+exp+reduce run on device).
"""

import numpy as np
import ml_dtypes

GAMMA = 1.0
N_QUERY, N_TRAIN, D = 8192, 8192, 512
N_CORES = 8
P = 128
QPC = N_QUERY // N_CORES  # 1024 queries per core
KS = D // P               # 4 contraction subtiles (d = ks*128 + p)
NQC = QPC // P            # 8 query chunks of 128
GT = 2048                 # train columns per psum group (4 banks)
NG = N_TRAIN // GT        # 4 groups
ST = 512                  # train cols per matmul (one psum bank)
NS = GT // ST             # 4 subtiles per group
# (g, c) tiles whose reduction runs on ScalarE via ACT accum_out.  Host
# sign-sorts train points (negatives first, majority negative after flip),
# so g=0/g=1 tiles are all-negative and g=3 tiles all-positive; g=2 holds
# the sign boundary and must reduce on DVE (sign-row multiply).  The g=3
# entries cover the last-processed tiles so the kernel doesn't end on a
# trailing DVE op.
SCC_NEG = ((0, 2), (0, 6))
SCC_POS = ((3, 4), (3, 5), (3, 6), (3, 7))
SCC = SCC_NEG + SCC_POS

_CACHE = {}


def _build_program():
    from contextlib import ExitStack

    import concourse.bass as bass
    import concourse.mybir as mybir
    import concourse.tile as tile
    from concourse import bacc

    f32 = mybir.dt.float32
    bf16 = mybir.dt.bfloat16
    f8 = mybir.dt.float8e4
    AF = mybir.ActivationFunctionType
    MUL = mybir.AluOpType.mult
    DR = mybir.MatmulPerfMode.DoubleRow

    nc = bacc.Bacc(
        "TRN2", target_bir_lowering=False, debug=False, num_devices=N_CORES
    )

    tt_d = nc.dram_tensor("tt_fp8", [D, N_TRAIN], f8, kind="ExternalInput").ap()
    x_d = nc.dram_tensor("x_fp8", [D, QPC], f8, kind="ExternalInput").ap()
    wb_d = nc.dram_tensor("wb_bf16", [P, N_TRAIN], bf16, kind="ExternalInput").ap()
    r8_d = nc.dram_tensor("r8_fp8", [1, N_TRAIN], f8, kind="ExternalInput").ap()
    x2_d = nc.dram_tensor("x2n_f32", [P, 2 * NQC], f32, kind="ExternalInput").ap()
    out_d = nc.dram_tensor("out", [QPC], f32, kind="ExternalOutput").ap()

    with tile.TileContext(nc) as tc, ExitStack() as ctx:
        res = ctx.enter_context(tc.tile_pool(name="res", bufs=1))
        etp = ctx.enter_context(tc.tile_pool(name="etp", bufs=4))
        scrp = ctx.enter_context(tc.tile_pool(name="scrp", bufs=4))
        psq = ctx.enter_context(tc.tile_pool(name="psq", bufs=2, space="PSUM"))

        x_sb = res.tile([P, KS, QPC], f8, tag="x")
        x2_sb = res.tile([P, 2 * NQC], f32, tag="x2")
        wb_sb = res.tile([P, N_TRAIN], bf16, tag="wb")
        r8_sb = res.tile([P, N_TRAIN], f8, tag="r8")
        tt_sb = res.tile([P, KS, N_TRAIN], f8, tag="tt")
        Scol = res.tile([P, NQC * NG], f32, tag="Scol")
        Acol = res.tile([P, max(1, len(SCC))], f32, tag="Acol")
        out_sb = res.tile([P, NQC], f32, tag="out")
        warm_sb = res.tile([P, 2, 128], f8, tag="warm")
        eights = res.tile([P, P], f8, tag="eights")
        ghost = res.tile([P, GT], bf16, tag="ghost")

        # loads split across the two hwdge queues (Sync + ScalarE); the
        # non-critical streams get wait_until floors so the first tiles'
        # operands (~1.6MB) get the full DMA bandwidth instead of sharing
        # it with 5MB of later-needed data
        nc.scalar.dma_start(x2_sb[:], x2_d[:])
        nc.scalar.dma_start(
            x_sb[:, :, 0:P], x_d[:, 0:P].rearrange("(k p) q -> p k q", k=KS)
        )
        nc.scalar.dma_start(
            x_sb[:, :, P:], x_d[:, P:].rearrange("(k p) q -> p k q", k=KS)
        )
        for b in range(NS):
            nc.scalar.dma_start(r8_sb[32 * b : 32 * b + 1, :], r8_d[:])
        nc.sync.dma_start(
            tt_sb[:, 0:2, 0:GT],
            tt_d[0 : 2 * P, 0:GT].rearrange("(k p) t -> p k t", k=2),
        )
        nc.sync.dma_start(
            tt_sb[:, 2:4, 0:GT],
            tt_d[2 * P :, 0:GT].rearrange("(k p) t -> p k t", k=2),
        )
        with tc.tile_wait_until(0.012):
            nc.sync.dma_start(wb_sb[:, 0:GT], wb_d[:, 0:GT])
        with tc.tile_wait_until(0.013):
            nc.sync.dma_start(
                tt_sb[:, :, GT : 2 * GT],
                tt_d[:, GT : 2 * GT].rearrange("(k p) t -> p k t", k=KS),
            )
        with tc.tile_wait_until(0.017):
            nc.sync.dma_start(wb_sb[:, GT : 2 * GT], wb_d[:, GT : 2 * GT])
        with tc.tile_wait_until(0.020):
            nc.sync.dma_start(
                tt_sb[:, :, 2 * GT : 3 * GT],
                tt_d[:, 2 * GT : 3 * GT].rearrange("(k p) t -> p k t", k=KS),
            )
        with tc.tile_wait_until(0.028):
            nc.sync.dma_start(
                tt_sb[:, :, 3 * GT :],
                tt_d[:, 3 * GT :].rearrange("(k p) t -> p k t", k=KS),
            )
            nc.sync.dma_start(wb_sb[:, 2 * GT :], wb_d[:, 2 * GT :])

        nc.vector.memset(warm_sb[:], 0)
        nc.vector.memset(eights[:], 8.0)
        nc.vector.memset(Scol[:], 0.0)
        # preload the exp table while DMAs stream (first real ACT would
        # otherwise pay the ~2.7us ACT_TABLE_LOAD on the critical path)
        warm_act = res.tile([P, 1], bf16, tag="wact")
        nc.scalar.activation(warm_act[:], warm_sb[:, 0, 0:1], AF.Exp, scale=1.0)

        # HAM warmup: keep the PE busy while the first train tiles stream in,
        # so the clock gate is at 8/8 when the real matmuls start.  Results
        # land in the first psum tile's banks and are discarded by the real
        # accumulation groups' start=True.
        ps0 = psq.tile([P, GT], f32, tag="ps")
        for r in range(36):
            nc.tensor.matmul(
                ps0[:, 0:128],
                warm_sb[:],
                warm_sb[:],
                start=True,
                stop=True,
                perf_mode=DR,
                skip_group_check=True,
            )

        first = True
        for g in range(NG):
            for c in range(NQC):
                ps = ps0 if first else psq.tile([P, GT], f32, tag="ps")
                first = False
                sc_tile = (g, c) in SCC
                for s in range(NS):
                    for j in range(KS // 2):
                        nc.tensor.matmul(
                            ps[:, s * ST : (s + 1) * ST],
                            x_sb[:, 2 * j : 2 * j + 2, c * P : (c + 1) * P],
                            tt_sb[:, 2 * j : 2 * j + 2, g * GT + s * ST : g * GT + (s + 1) * ST],
                            start=(j == 0),
                            stop=(j == KS // 2 - 1) and not sc_tile,
                            perf_mode=DR,
                        )
                if sc_tile:
                    # ScalarE-reduced tile: add the t-varying ln|w[t]| - B row
                    # via 4 concurrently-packed K=1 matmuls (distinct 32-row
                    # strips, distinct psum banks): ps[:, bank s] += 8*(r/8)
                    for s in range(NS):
                        nc.tensor.matmul(
                            ps[:, s * ST : (s + 1) * ST],
                            eights[32 * s : 32 * s + 1, :],
                            r8_sb[32 * s : 32 * s + 1, g * GT + s * ST : g * GT + (s + 1) * ST],
                            start=False,
                            stop=True,
                            tile_position=(32 * s, 0),
                        )
                    # sign-pure tile: ScalarE reduces it for free via accum
                    nc.scalar.activation(
                        ghost[:],
                        ps[:],
                        AF.Exp,
                        bias=x2_sb[:, NQC + c : NQC + c + 1],
                        scale=2.0 * GAMMA,
                        accum_out=Acol[:, SCC.index((g, c)) : SCC.index((g, c)) + 1],
                    )
                else:
                    et = etp.tile([P, GT], bf16, tag="et")
                    nc.scalar.activation(
                        et[:], ps[:], AF.Exp, bias=x2_sb[:, c : c + 1], scale=2.0 * GAMMA
                    )
                    scr = scrp.tile([P, GT], bf16, tag="scr")
                    nc.vector.scalar_tensor_tensor(
                        scr[:],
                        et[:],
                        1.0,
                        wb_sb[:, g * GT : (g + 1) * GT],
                        MUL,
                        MUL,
                        accum_out=Scol[:, c * NG + g : c * NG + g + 1],
                    )
        for c in range(NQC):
            nc.vector.tensor_reduce(
                out_sb[:, c : c + 1],
                Scol[:, c * NG : (c + 1) * NG],
                axis=mybir.AxisListType.X,
                op=mybir.AluOpType.add,
            )
            for (g2, c2) in SCC_NEG:
                if c2 == c:
                    # all-negative-coef tile: subtract its ScalarE accum
                    nc.vector.tensor_tensor(
                        out_sb[:, c : c + 1],
                        out_sb[:, c : c + 1],
                        Acol[:, SCC.index((g2, c2)) : SCC.index((g2, c2)) + 1],
                        mybir.AluOpType.subtract,
                    )
            for (g2, c2) in SCC_POS:
                if c2 == c:
                    nc.vector.tensor_tensor(
                        out_sb[:, c : c + 1],
                        out_sb[:, c : c + 1],
                        Acol[:, SCC.index((g2, c2)) : SCC.index((g2, c2)) + 1],
                        mybir.AluOpType.add,
                    )
        # p-major out layout: per-partition contiguous 32B runs instead of
        # 1024 scattered 4B descriptors; kernel() un-permutes on host
        nc.sync.dma_start(out_d.rearrange("(p c) -> p c", p=P), out_sb[:])

    nc.compile()
    return nc


def _get_program():
    if "nc" not in _CACHE:
        _CACHE["nc"] = _build_program()
    return _CACHE["nc"]


def make_in_maps(X, train_X, dual_coef):
    bf = ml_dtypes.bfloat16
    f8 = ml_dtypes.float8_e4m3

    X = np.asarray(X, dtype=np.float32)
    train_X = np.asarray(train_X, dtype=np.float32)
    dual_coef = np.asarray(dual_coef, dtype=np.float32)

    # flip so negative coefs are the majority (>= 4096 >= GT); the host
    # negates the final output back.  Then sort negatives first so the
    # g=0 tiles (first GT columns) are sign-pure for the ScalarE reduction.
    flip = (dual_coef < 0).sum() < N_TRAIN // 2
    coef = -dual_coef if flip else dual_coef
    perm = np.concatenate([np.where(coef < 0)[0], np.where(coef >= 0)[0]])
    coef = coef[perm]
    train_s = train_X[perm]

    ttq = np.ascontiguousarray(train_s.T).astype(f8)          # [D, N_TRAIN]
    y2 = np.einsum("td,td->t", train_s, train_s)              # [N_TRAIN]
    lnw = -GAMMA * y2 + np.log(np.maximum(np.abs(coef), 1e-30))
    B = float(np.mean(lnw))
    # ACT applies scale=2g to the whole psum (dot + r-row), so pre-divide
    # r by 2g; /8 pairs with the constant-8 stationary of the bias matmuls
    r8 = ((lnw - B) / (2.0 * GAMMA) / 8.0).astype(f8).reshape(1, N_TRAIN)
    w = (np.exp(-GAMMA * y2) * coef).astype(bf)               # DVE-tile weights
    wbb = np.ascontiguousarray(np.broadcast_to(w[None, :], (P, N_TRAIN)))
    x2 = np.einsum("qd,qd->q", X, X)                          # [N_QUERY]
    XT = np.ascontiguousarray(X.T)                            # [D, N_QUERY]

    in_maps = []
    for c in range(N_CORES):
        xs = np.ascontiguousarray(XT[:, c * QPC : (c + 1) * QPC]).astype(f8)
        x2n = -GAMMA * x2[c * QPC : (c + 1) * QPC]
        x2c = np.ascontiguousarray(
            np.concatenate(
                [x2n.reshape(NQC, P).T, (x2n + B).reshape(NQC, P).T], axis=1
            ).astype(np.float32)
        )
        in_maps.append(
            {
                "tt_fp8": ttq,
                "x_fp8": xs,
                "wb_bf16": wbb,
                "r8_fp8": r8,
                "x2n_f32": x2c,
            }
        )
    return in_maps, flip


def _get_callable():
    """Cached (fn, in_names, out_names, out_avals, zero_outs, mesh) for the
    sharded 8-core NEFF execution."""
    if "call" in _CACHE:
        return _CACHE["call"]

    import jax
    from jax.sharding import Mesh, PartitionSpec
    from jax.experimental.shard_map import shard_map

    import concourse.mybir as mybir
    from concourse import bass2jax
    from concourse.bass2jax import install_neuronx_cc_hook

    install_neuronx_cc_hook()
    nc = _get_program()

    partition_name = (
        nc.partition_id_tensor.name if nc.partition_id_tensor else None
    )
    in_names, out_names, out_avals, zero_outs = [], [], [], []
    for alloc in nc.m.functions[0].allocations:
        if not isinstance(alloc, mybir.MemoryLocationSet):
            continue
        if alloc.kind not in ("ExternalInput", "ExternalOutput"):
            continue
        name = alloc.memorylocations[0].name
        if alloc.kind == "ExternalInput":
            if name != partition_name:
                in_names.append(name)
        else:
            out_names.append(name)
            shape = tuple(alloc.tensor_shape)
            dtype = mybir.dt.np(alloc.dtype)
            out_avals.append(jax.core.ShapedArray(shape, dtype))
            zero_outs.append(np.zeros(shape, dtype))
    all_in_names = in_names + out_names
    if partition_name is not None:
        all_in_names = all_in_names + [partition_name]

    def _body(*args):
        operands = list(args)
        if partition_name is not None:
            operands.append(bass2jax.partition_id_tensor())
        outs = bass2jax._bass_exec_p.bind(
            *operands,
            out_avals=tuple(out_avals),
            in_names=tuple(all_in_names),
            out_names=tuple(out_names),
            lowering_input_output_aliases=(),
            sim_require_finite=True,
            sim_require_nnan=True,
            nc=nc,
        )
        return tuple(outs)

    devices = jax.devices()[:N_CORES]
    mesh = Mesh(np.asarray(devices), ("core",))
    n_all = len(in_names) + len(out_names)
    fn = jax.jit(
        shard_map(
            _body,
            mesh=mesh,
            in_specs=(PartitionSpec("core"),) * n_all,
            out_specs=(PartitionSpec("core"),) * len(out_names),
            check_rep=False,
        ),
        keep_unused=True,
    )
    _CACHE["call"] = (fn, in_names, out_names, out_avals, zero_outs, mesh)
    return _CACHE["call"]


def concat_inputs(in_maps):
    fn, in_names, out_names, out_avals, zero_outs, mesh = _get_callable()
    concat_in = [
        np.concatenate([np.asarray(m[name]) for m in in_maps], axis=0)
        for name in in_names
    ]
    concat_zeros = [
        np.zeros((N_CORES * z.shape[0], *z.shape[1:]), z.dtype) for z in zero_outs
    ]
    return concat_in + concat_zeros


def kernel(X, train_X, dual_coef):
    X = np.asarray(X, dtype=np.float32)
    train_X = np.asarray(train_X, dtype=np.float32)
    dual_coef = np.asarray(dual_coef, dtype=np.float32)

    fn, in_names, out_names, out_avals, zero_outs, mesh = _get_callable()
    in_maps, flip = make_in_maps(X, train_X, dual_coef)
    args = concat_inputs(in_maps)
    outs = fn(*args)
    out = np.asarray(outs[0]).reshape(N_CORES, P, NQC)
    # device wrote p-major ([p, c] with q = c*128 + p); un-permute per core
    out = out.transpose(0, 2, 1).reshape(-1)
    if flip:
        out = -out
    return np.ascontiguousarray(out).astype(np.float32)
